# revision 30
# baseline (speedup 1.0000x reference)
"""GroupedQueryAttention TRN2 Bass kernel, sharded over 8 NeuronCores.

Problem (hardcoded): B=2, T=2048, D=4096, 32 Q heads x 128, 8 KV groups x 128,
RoPE (base 5e5), causal, out = ctx @ Wo.

Sharding: core g owns Q heads 4g..4g+3 (Wq columns 512g:512g+512), KV group g
(Wk/Wv columns 128g:128g+128), and Wo rows 512g:512g+512 (row-parallel).
Each core computes a full-shape partial output (bf16); host sums the 8
partials in fp32.

Matmuls run in float32r (fp32 with 11-bit mantissa, full PE rate); the
attention context and output projection run in bf16 (same PE rate, half the
SBUF/DMA traffic). Inputs are pre-rounded to fp32r on host.

v2 versus baseline:
 - softmax reciprocal via reciprocal_approx_fast (was 3.3us serial DVE
   reciprocal per q-tile, stalling the PE every tile)
 - causal diagonal blocks narrowed: score/exp/AV/rowsum matmuls only cover
   valid query columns (N = 512-128r); single [128,128] triangle mask
 - ctx kept in SBUF as bf16 (no DRAM round-trip), Wo in bf16, output
   partials written bf16 (halves phase-C DMA; one 1MB DMA per token tile)
 - exp ACT table preloaded at kernel start; Wo prefetched during attention
"""
import sys
import numpy as np

for _p in ("/opt/trn_rl_repo", "/root/.axon_site", "/root/.axon_site/_ro/trn_rl_repo"):
    if _p not in sys.path:
        sys.path.append(_p)

from contextlib import ExitStack

import concourse.bass as bass
import concourse.tile as tile
from concourse import bacc, mybir
from concourse.bass_utils import run_bass_kernel_spmd
from concourse.masks import make_identity

B, T, D = 2, 2048, 4096
NH, NKV, DH = 32, 8, 128
HPC = NH // 8          # 4 q heads per core
FPC = HPC * DH         # 512 q features per core
ROPE_BASE = 500000.0
NT = B * T             # 4096 tokens
f32 = mybir.dt.float32
f32r = mybir.dt.float32r
bf16 = mybir.dt.bfloat16
EXP_SCALE = 1.0 / float(np.sqrt(DH))

_NC_CACHE = {}


def _round_fp32r(x):
    x = np.ascontiguousarray(x, dtype=np.float32)
    u = x.view(np.uint32)
    lsb = (u >> 12) & np.uint32(1)
    r = (u + np.uint32(0x7FF) + lsb) & np.uint32(0xFFFFF000)
    return r.view(np.float32)


def _build_program():
    nc = bacc.Bacc("TRN2", target_bir_lowering=False, debug=False)

    xT = nc.dram_tensor("xT", [D, NT], bf16, kind="ExternalInput").ap()
    wq = nc.dram_tensor("wq", [D, FPC], bf16, kind="ExternalInput").ap()
    wk = nc.dram_tensor("wk", [D, DH], bf16, kind="ExternalInput").ap()
    wv = nc.dram_tensor("wv", [D, DH], bf16, kind="ExternalInput").ap()
    wo = nc.dram_tensor("wo", [FPC, D], bf16, kind="ExternalInput").ap()
    ropeA = nc.dram_tensor("ropeA", [128, T], f32, kind="ExternalInput").ap()
    ropeB = nc.dram_tensor("ropeB", [128, T], f32, kind="ExternalInput").ap()
    trid = nc.dram_tensor("trid", [128, 128], f32r, kind="ExternalInput").ap()
    outp = nc.dram_tensor("outp", [NT, D], bf16, kind="ExternalOutput").ap()

    qTd = nc.dram_tensor("qTd", [FPC, NT], f32r).ap()     # roped Q^T, feature-major

    KC = D // 128  # 32 contraction chunks

    with tile.TileContext(nc) as tc, ExitStack() as s0:
        kvp = s0.enter_context(tc.tile_pool(name="kv", bufs=1))
        KTb = [kvp.tile([128, T], f32r, tag=f"KT{i}", name=f"KT{i}") for i in range(B)]
        Vb = [kvp.tile([128, T], f32r, tag=f"V{i}", name=f"V{i}") for i in range(B)]
        ident_f = kvp.tile([128, 128], f32, tag="ident_f")
        make_identity(nc, ident_f[:])
        ident = kvp.tile([128, 128], f32r, tag="ident")
        nc.vector.tensor_copy(ident[:], ident_f[:])
        ones_f = kvp.tile([128, 1], f32, tag="ones_f")
        nc.vector.memset(ones_f[:], 1.0)
        ones = kvp.tile([128, 1], f32r, tag="ones")
        nc.vector.tensor_copy(ones[:], ones_f[:])
        ones_row_f = kvp.tile([1, 128], f32, tag="ones_row_f")
        nc.vector.memset(ones_row_f[:], 1.0)
        ones_row = kvp.tile([1, 128], f32r, tag="ones_row")
        nc.vector.tensor_copy(ones_row[:], ones_row_f[:])
        tri_sb = kvp.tile([128, 128], f32r, tag="tri")
        # first Q-head tile, prefetched during phase A (n==3) so attention
        # starts without a DMA stall
        qt0 = kvp.tile([128, T], f32r, tag="qt0")
        # preload the exp table set so the first attention exp doesn't pay
        # the ~2.7us ACT_TABLE_LOAD mid-kernel
        warm = kvp.tile([1, 2], f32, tag="warm")
        nc.vector.memset(warm[:], 0.0)
        nc.scalar.activation(warm[0:1, 0:1], warm[0:1, 1:2],
                             mybir.ActivationFunctionType.Exp, scale=1.0)

        # ---------------- Phase A: projections + RoPE -----------------
        # Eviction/rope tiles for the LAST n-tile live in a pool that stays
        # open to kernel end: phase B's pools then never wait on the final
        # rope chain (the A-scoped pools' releases would otherwise serialize
        # phase B behind ~9us of trailing DVE work).
        evt = s0.enter_context(tc.tile_pool(name="evt", bufs=1))
        with ExitStack() as sa:
            wp = sa.enter_context(tc.tile_pool(name="wts", bufs=1))
            wq_sb = wp.tile([128, KC * FPC], bf16, tag="wq")
            wk_sb = wp.tile([128, KC * DH], bf16, tag="wk")
            wv_sb = wp.tile([128, KC * DH], bf16, tag="wv")
            # rope tables in the never-released pool: the last n-tile's rope
            # reads them, and they must not pin the weight pool's release
            tabA = evt.tile([128, T], f32, tag="tabA")
            tabB = evt.tile([128, T], f32, tag="tabB")

            SLAB = 4
            NSLAB = KC // SLAB

            def load_w_slab(s):
                # one batched 3D-AP DMA per weight: DMA *issue* time on the
                # Sync queue (~0.6us each) is the startup bottleneck, not HBM.
                # Slab 0 splits out chunk 0 of Wq so the first matmul can
                # start as soon as ~0.25MB (vs ~2.3MB) has landed.
                k0, k1 = s * SLAB, (s + 1) * SLAB
                if s == 0:
                    nc.sync.dma_start(wq_sb[:, 0:FPC], wq[0:128, :])
                    nc.sync.dma_start(
                        wq_sb[:, FPC:k1 * FPC],
                        wq[128:k1 * 128, :].rearrange("(j p) c -> p j c", p=128))
                else:
                    nc.sync.dma_start(
                        wq_sb[:, k0 * FPC:k1 * FPC],
                        wq[k0 * 128:k1 * 128, :].rearrange("(j p) c -> p j c", p=128))
                nc.sync.dma_start(
                    wk_sb[:, k0 * DH:k1 * DH],
                    wk[k0 * 128:k1 * 128, :].rearrange("(j p) c -> p j c", p=128))
                nc.sync.dma_start(
                    wv_sb[:, k0 * DH:k1 * DH],
                    wv[k0 * 128:k1 * 128, :].rearrange("(j p) c -> p j c", p=128))

            xsp = sa.enter_context(tc.tile_pool(name="xs", bufs=3))
            evp = sa.enter_context(tc.tile_pool(name="ev", bufs=1))
            psA = sa.enter_context(tc.tile_pool(name="psA", bufs=1, space="PSUM"))

            def stationary(m, k):
                if m < HPC:
                    return wq_sb[:, k * FPC + m * 128: k * FPC + (m + 1) * 128]
                if m == HPC:
                    return wk_sb[:, k * DH:(k + 1) * DH]
                return wv_sb[:, k * DH:(k + 1) * DH]

            pending_vt = None

            def flush_vt():
                nonlocal pending_vt
                if pending_vt is None:
                    return
                vt_p, n_p = pending_vt
                b_p = n_p // 4
                for i in range(4):
                    ptr = psA.tile([128, 128], f32r, tag="tr", bufs=2, name="ptr")
                    nc.tensor.transpose(ptr[:], vt_p[:, i * 128:(i + 1) * 128], ident[:])
                    c_local = 4 * (n_p % 4) + i
                    nc.scalar.copy(Vb[b_p][:, c_local * 128:c_local * 128 + 128], ptr[:])
                pending_vt = None

            for n in range(NT // 512):
                b, tloc = n // 4, 512 * (n % 4)
                ps = [psA.tile([128, 512], f32, tag=f"ps{m}", name=f"ps{m}")
                      for m in range(6)]
                for s in range(NSLAB):
                    if n == 0:
                        load_w_slab(s)
                    xsl = xsp.tile([128, SLAB * 512], bf16, tag="xs", name="xsl")
                    if n == 0 and s == 0:
                        nc.sync.dma_start(xsl[:, 0:512], xT[0:128, 0:512])
                        nc.sync.dma_start(
                            xsl[:, 512:],
                            xT[128:SLAB * 128, 0:512].rearrange(
                                "(j p) c -> p j c", p=128))
                        # deferred setup loads: not needed until ~45us in
                        nc.sync.dma_start(tri_sb[:], trid)
                        nc.sync.dma_start(tabA[:], ropeA)
                        nc.sync.dma_start(tabB[:], ropeB)
                    else:
                        nc.sync.dma_start(
                            xsl[:],
                            xT[s * SLAB * 128:(s + 1) * SLAB * 128,
                               n * 512:(n + 1) * 512].rearrange(
                                   "(j p) c -> p j c", p=128))
                    for m in range(6):
                        for j in range(SLAB):
                            k = s * SLAB + j
                            nc.tensor.matmul(ps[m][:], stationary(m, k),
                                             xsl[:, j * 512:(j + 1) * 512],
                                             start=(k == 0), stop=(k == KC - 1))
                    if s == 0:
                        flush_vt()   # prev n-tile's V transposes, PE already warm
                # evict: ACT copies free PSUM banks at ACT pace; on the last
                # n-tile, flush V first and split evictions across ACT/DVE so
                # phase B's PSUM banks free as early as possible
                last = n == NT // 512 - 1
                pool = evt if last else evp
                qes = []
                if last:
                    vt = pool.tile([128, 512], f32r, tag="vt", bufs=1, name="vt")
                    nc.scalar.copy(vt[:], ps[5][:])
                    pending_vt = (vt, n)
                    flush_vt()
                    for m in range(5):
                        qe = pool.tile([128, 512], f32, tag="qe", bufs=5, name=f"qe{m}")
                        if m % 2 == 1:
                            nc.vector.tensor_copy(qe[:], ps[m][:])
                        else:
                            nc.scalar.copy(qe[:], ps[m][:])
                        qes.append(qe)
                else:
                    for m in range(5):
                        qe = pool.tile([128, 512], f32, tag="qe", bufs=6, name=f"qe{m}")
                        nc.scalar.copy(qe[:], ps[m][:])
                        qes.append(qe)
                    vt = pool.tile([128, 512], f32r, tag="vt", bufs=2, name="vt")
                    nc.scalar.copy(vt[:], ps[5][:])
                    pending_vt = (vt, n)
                # rope chains on DVE
                tA = tabA[:, tloc:tloc + 512]
                tB = tabB[:, tloc:tloc + 512]
                for m in range(5):
                    qe = qes[m]
                    sw = pool.tile([128, 512], f32, tag="sw", bufs=1, name="sw")
                    nc.vector.tensor_copy(sw[0:64, :], qe[64:128, :])
                    nc.vector.tensor_copy(sw[64:128, :], qe[0:64, :])
                    mm = pool.tile([128, 512], f32, tag="mm", bufs=1, name="mm")
                    nc.vector.tensor_mul(mm[:], sw[:], tB)
                    tt = pool.tile([128, 512], f32, tag="tt", bufs=1, name="tt")
                    nc.vector.tensor_mul(tt[:], qe[:], tA)
                    if m < HPC:
                        ro = pool.tile([128, 512], f32r, tag="ro", bufs=2, name="ro")
                        nc.vector.tensor_add(ro[:], tt[:], mm[:])
                        nc.sync.dma_start(qTd[m * 128:(m + 1) * 128, n * 512:(n + 1) * 512], ro[:])
                    else:
                        nc.vector.tensor_add(KTb[b][:, tloc:tloc + 512], tt[:], mm[:])
                if n == 3:
                    # qTd rows for head 0 of batch 0 are complete: prefetch
                    # the first attention Q tile while phase A continues
                    nc.sync.dma_start(qt0[:], qTd[0:128, 0:T])

        # ---------------- Phase B: attention (ctx -> SBUF bf16) --------
        # Pool-open order controls space reuse (stack allocator): the
        # attention-critical pools (sxp/smp/qtp) open first so they land in
        # the weight pool's space, whose release only waits on phase A's
        # matmuls. ctx/wo open after and overlap the eviction pool, whose
        # release waits on the final rope DMAs — but their first use is well
        # into phase B. All SBUF pools stay open until kernel end (no stack
        # pops mid-kernel); only psB releases before phase C's PSUM pool.
        sxp = s0.enter_context(tc.tile_pool(name="sxp", bufs=6))
        smp = s0.enter_context(tc.tile_pool(name="smp", bufs=2))
        qtp = s0.enter_context(tc.tile_pool(name="qtp", bufs=2))
        ctxp = s0.enter_context(tc.tile_pool(name="ctxp", bufs=1))
        ctx_sb = [ctxp.tile([128, NT], bf16, tag=f"ctx{h}", name=f"ctx{h}")
                  for h in range(HPC)]
        wop = s0.enter_context(tc.tile_pool(name="wop", bufs=1))
        wo_sb = wop.tile([128, HPC * D], bf16, tag="wo")

        # psB split: the normalize tiles (ctx accumulator + broadcast bank)
        # outlive the score/rowsum banks, so phase C's PSUM pool can allocate
        # as soon as the main attention banks release.
        psBt = s0.enter_context(tc.tile_pool(name="psBt", bufs=2, space="PSUM"))
        with ExitStack() as sbc:
            psB = sbc.enter_context(tc.tile_pool(name="psB", bufs=2, space="PSUM"))

            pending_norm = None

            def flush_norm():
                nonlocal pending_norm
                if pending_norm is None:
                    return
                ps_ctx_p, rs_p, h_p, b_p, qt_p = pending_norm
                # broadcast rs across partitions: ones_row^T @ rs (K=1 matmul)
                ps_bc = psBt.tile([128, 512], f32, tag="bc", bufs=1, name="ps_bc")
                nc.tensor.matmul(ps_bc[:], ones_row[:], rs_p[:], start=True, stop=True)
                bcs = smp.tile([128, 512], f32, tag="bcs", name="bcs")
                nc.vector.tensor_copy(bcs[:], ps_bc[:])
                nc.vector.tensor_mul(
                    ctx_sb[h_p][:, b_p * T + qt_p * 512: b_p * T + (qt_p + 1) * 512],
                    ps_ctx_p[:], bcs[:])
                pending_norm = None

            for b in range(B):
                for h in range(HPC):
                    if b == 0 and h == 0:
                        qt_t = qt0
                    else:
                        qt_t = qtp.tile([128, T], f32r, tag="qt")
                        nc.sync.dma_start(qt_t[:], qTd[h * 128:(h + 1) * 128, b * T:(b + 1) * T])
                    if b == 0:
                        # prefetch one Wo head-slab per head; space frees as
                        # phase A's weight pool closes
                        nc.sync.dma_start(wo_sb[:, h * D:(h + 1) * D],
                                          wo[h * 128:(h + 1) * 128, :])
                    for qt in range(4):
                        ps_ctx = psBt.tile([128, 512], f32, tag="ctx")
                        ps_sm = psB.tile([1, 512], f32, tag="sm")
                        nk = 4 * qt + 4

                        def issue_st(kt):
                            off = max(0, (kt - 4 * qt) * 128)
                            ps_st = psB.tile([128, 512], f32, tag="st", bufs=3, name="ps_st")
                            nc.tensor.matmul(ps_st[:, off:],
                                             KTb[b][:, kt * 128:(kt + 1) * 128],
                                             qt_t[:, qt * 512 + off:(qt + 1) * 512],
                                             start=True, stop=True)
                            se = sxp.tile([128, 512], f32r, tag="se", name="se")
                            nc.scalar.activation(se[:, off:], ps_st[:, off:],
                                                 mybir.ActivationFunctionType.Exp,
                                                 scale=EXP_SCALE)
                            if kt >= 4 * qt:
                                nc.vector.tensor_mul(se[:, off:off + 128],
                                                     se[:, off:off + 128], tri_sb[:])
                            return se, off

                        se_q = [issue_st(0), issue_st(1)]
                        for kt in range(nk):
                            se_cur, off = se_q.pop(0)
                            if kt + 2 < nk:
                                se_q.append(issue_st(kt + 2))
                            nc.tensor.matmul(ps_ctx[:, off:],
                                             Vb[b][:, kt * 128:(kt + 1) * 128],
                                             se_cur[:, off:],
                                             start=(kt == 0), stop=(kt == nk - 1))
                            nc.tensor.matmul(ps_sm[0:1, off:], ones[:], se_cur[:, off:],
                                             start=(kt == 0), stop=(kt == nk - 1))
                            if kt == 2:
                                flush_norm()  # prev q-tile's normalize, PE already busy
                        # reciprocal immediately (fast approx), consume one tile later
                        rs_f = smp.tile([1, 512], f32, tag="rsf", name="rs_f")
                        nc.vector.reciprocal_approx_fast(out=rs_f[:], in_=ps_sm[:])
                        rs = smp.tile([1, 512], f32r, tag="rs", name="rs")
                        nc.vector.tensor_copy(rs[:], rs_f[:])
                        pending_norm = (ps_ctx, rs, h, b, qt)

        # psB (score/rowsum banks) released here; the final q-tile's
        # normalize uses only psBt and runs concurrently with phase C's start
        flush_norm()

        # ---------------- Phase C: output projection (SBUF bf16) ----
        with ExitStack() as sc:
            psC = sc.enter_context(tc.tile_pool(name="psC", bufs=4, space="PSUM"))
            obp = sc.enter_context(tc.tile_pool(name="obp", bufs=2))
            for m in range(NT // 128):
                ob = obp.tile([128, D], bf16, tag="ob")
                for n in range(D // 512):
                    pso = psC.tile([128, 512], f32, tag="oc")
                    for h in range(HPC):
                        nc.tensor.matmul(pso[:],
                                         ctx_sb[h][:, m * 128:(m + 1) * 128],
                                         wo_sb[:, h * D + n * 512: h * D + (n + 1) * 512],
                                         start=(h == 0), stop=(h == HPC - 1))
                    # alternate eviction engines so neither stalls the PE
                    if n % 2 == 0:
                        nc.scalar.copy(ob[:, n * 512:(n + 1) * 512], pso[:])
                    else:
                        nc.vector.tensor_copy(ob[:, n * 512:(n + 1) * 512], pso[:])
                    if m == NT // 128 - 1 and n % 2 == 1:
                        # split the final tile's output DMA so the kernel tail
                        # isn't one serial 1MB transfer after the last eviction
                        nc.sync.dma_start(
                            outp[m * 128:(m + 1) * 128, (n - 1) * 512:(n + 1) * 512],
                            ob[:, (n - 1) * 512:(n + 1) * 512])
                if m < NT // 128 - 1:
                    nc.sync.dma_start(outp[m * 128:(m + 1) * 128, :], ob[:])

    nc.compile()
    return nc


def _get_nc():
    if "nc" not in _NC_CACHE:
        _NC_CACHE["nc"] = _build_program()
    return _NC_CACHE["nc"]


def _rope_tables():
    j = np.arange(0, DH, 2, dtype=np.float32) / np.float32(DH)
    inv_freq = (np.float32(1.0) / (np.float32(ROPE_BASE) ** j)).astype(np.float32)
    t = np.arange(T, dtype=np.float32)
    freqs = np.outer(t, inv_freq).astype(np.float32)   # (T, 64)
    c = np.cos(freqs).astype(np.float32).T             # (64, T)
    s = np.sin(freqs).astype(np.float32).T
    A = np.vstack([c, c]).astype(np.float32)           # (128, T)
    Bt = np.vstack([-s, s]).astype(np.float32)
    return np.ascontiguousarray(A), np.ascontiguousarray(Bt)


def _tri_mask():
    p = np.arange(128)[:, None]
    f = np.arange(128)[None, :]
    return np.ascontiguousarray((p <= f).astype(np.float32))


def _build_in_maps(x, Wq, Wk, Wv, Wo):
    import ml_dtypes

    bf = ml_dtypes.bfloat16
    xT = np.ascontiguousarray(x.reshape(NT, D).T).astype(bf)
    A, Bt = _rope_tables()
    tri = _tri_mask()
    in_maps = []
    for g in range(8):
        in_maps.append({
            "xT": xT,
            "wq": np.ascontiguousarray(Wq[:, g * FPC:(g + 1) * FPC]).astype(bf),
            "wk": np.ascontiguousarray(Wk[:, g * DH:(g + 1) * DH]).astype(bf),
            "wv": np.ascontiguousarray(Wv[:, g * DH:(g + 1) * DH]).astype(bf),
            "wo": np.ascontiguousarray(
                Wo[g * FPC:(g + 1) * FPC, :]).astype(bf),
            "ropeA": A,
            "ropeB": Bt,
            "trid": _round_fp32r(tri),
        })
    return in_maps


def kernel(x, Wq, Wk, Wv, Wo):
    x = np.asarray(x, dtype=np.float32)
    Wq = np.asarray(Wq, dtype=np.float32)
    Wk = np.asarray(Wk, dtype=np.float32)
    Wv = np.asarray(Wv, dtype=np.float32)
    Wo = np.asarray(Wo, dtype=np.float32)

    nc = _get_nc()
    in_maps = _build_in_maps(x, Wq, Wk, Wv, Wo)

    res = run_bass_kernel_spmd(nc, in_maps, list(range(8)))
    acc = res.results[0]["outp"].astype(np.float32)
    for g in range(1, 8):
        acc = acc + res.results[g]["outp"].astype(np.float32)
    return np.ascontiguousarray(acc.reshape(B, T, D), dtype=np.float32)


# revision 31
# speedup vs baseline: 1.0090x; 1.0090x over previous
"""GroupedQueryAttention TRN2 Bass kernel, sharded over 8 NeuronCores.

Problem (hardcoded): B=2, T=2048, D=4096, 32 Q heads x 128, 8 KV groups x 128,
RoPE (base 5e5), causal, out = ctx @ Wo.

Sharding: core g owns Q heads 4g..4g+3 (Wq columns 512g:512g+512), KV group g
(Wk/Wv columns 128g:128g+128), and Wo rows 512g:512g+512 (row-parallel).
Each core computes a full-shape partial output (bf16); host sums the 8
partials in fp32.

Matmuls run in float32r (fp32 with 11-bit mantissa, full PE rate); the
attention context and output projection run in bf16 (same PE rate, half the
SBUF/DMA traffic). Inputs are pre-rounded to fp32r on host.

v2 versus baseline:
 - softmax reciprocal via reciprocal_approx_fast (was 3.3us serial DVE
   reciprocal per q-tile, stalling the PE every tile)
 - causal diagonal blocks narrowed: score/exp/AV/rowsum matmuls only cover
   valid query columns (N = 512-128r); single [128,128] triangle mask
 - ctx kept in SBUF as bf16 (no DRAM round-trip), Wo in bf16, output
   partials written bf16 (halves phase-C DMA; one 1MB DMA per token tile)
 - exp ACT table preloaded at kernel start; Wo prefetched during attention
"""
import sys
import numpy as np

for _p in ("/opt/trn_rl_repo", "/root/.axon_site", "/root/.axon_site/_ro/trn_rl_repo"):
    if _p not in sys.path:
        sys.path.append(_p)

from contextlib import ExitStack

import concourse.bass as bass
import concourse.tile as tile
from concourse import bacc, mybir
from concourse.bass_utils import run_bass_kernel_spmd
from concourse.masks import make_identity

B, T, D = 2, 2048, 4096
NH, NKV, DH = 32, 8, 128
HPC = NH // 8          # 4 q heads per core
FPC = HPC * DH         # 512 q features per core
ROPE_BASE = 500000.0
NT = B * T             # 4096 tokens
f32 = mybir.dt.float32
f32r = mybir.dt.float32r
bf16 = mybir.dt.bfloat16
EXP_SCALE = 1.0 / float(np.sqrt(DH))

_NC_CACHE = {}


def _round_fp32r(x):
    x = np.ascontiguousarray(x, dtype=np.float32)
    u = x.view(np.uint32)
    lsb = (u >> 12) & np.uint32(1)
    r = (u + np.uint32(0x7FF) + lsb) & np.uint32(0xFFFFF000)
    return r.view(np.float32)


def _build_program():
    nc = bacc.Bacc("TRN2", target_bir_lowering=False, debug=False)

    xT = nc.dram_tensor("xT", [D, NT], bf16, kind="ExternalInput").ap()
    wq = nc.dram_tensor("wq", [D, FPC], bf16, kind="ExternalInput").ap()
    wk = nc.dram_tensor("wk", [D, DH], bf16, kind="ExternalInput").ap()
    wv = nc.dram_tensor("wv", [D, DH], bf16, kind="ExternalInput").ap()
    wo = nc.dram_tensor("wo", [FPC, D], bf16, kind="ExternalInput").ap()
    ropeA = nc.dram_tensor("ropeA", [128, T], f32, kind="ExternalInput").ap()
    ropeB = nc.dram_tensor("ropeB", [128, T], f32, kind="ExternalInput").ap()
    trid = nc.dram_tensor("trid", [128, 128], f32r, kind="ExternalInput").ap()
    outp = nc.dram_tensor("outp", [NT, D], bf16, kind="ExternalOutput").ap()

    qTd = nc.dram_tensor("qTd", [FPC, NT], f32r).ap()     # roped Q^T, feature-major

    KC = D // 128  # 32 contraction chunks

    with tile.TileContext(nc) as tc, ExitStack() as s0:
        kvp = s0.enter_context(tc.tile_pool(name="kv", bufs=1))
        KTb = [kvp.tile([128, T], f32r, tag=f"KT{i}", name=f"KT{i}") for i in range(B)]
        Vb = [kvp.tile([128, T], f32r, tag=f"V{i}", name=f"V{i}") for i in range(B)]
        ident_f = kvp.tile([128, 128], f32, tag="ident_f")
        make_identity(nc, ident_f[:])
        ident = kvp.tile([128, 128], f32r, tag="ident")
        nc.vector.tensor_copy(ident[:], ident_f[:])
        ones_f = kvp.tile([128, 1], f32, tag="ones_f")
        nc.vector.memset(ones_f[:], 1.0)
        ones = kvp.tile([128, 1], f32r, tag="ones")
        nc.vector.tensor_copy(ones[:], ones_f[:])
        ones_row_f = kvp.tile([1, 128], f32, tag="ones_row_f")
        nc.vector.memset(ones_row_f[:], 1.0)
        ones_row = kvp.tile([1, 128], f32r, tag="ones_row")
        nc.vector.tensor_copy(ones_row[:], ones_row_f[:])
        tri_sb = kvp.tile([128, 128], f32r, tag="tri")
        # first Q-head tile, prefetched during phase A (n==3) so attention
        # starts without a DMA stall
        qt0 = kvp.tile([128, T], f32r, tag="qt0")
        # preload the exp table set so the first attention exp doesn't pay
        # the ~2.7us ACT_TABLE_LOAD mid-kernel
        warm = kvp.tile([1, 2], f32, tag="warm")
        nc.vector.memset(warm[:], 0.0)
        nc.scalar.activation(warm[0:1, 0:1], warm[0:1, 1:2],
                             mybir.ActivationFunctionType.Exp, scale=1.0)

        # ---------------- Phase A: projections + RoPE -----------------
        # Eviction/rope tiles for the LAST n-tile live in a pool that stays
        # open to kernel end: phase B's pools then never wait on the final
        # rope chain (the A-scoped pools' releases would otherwise serialize
        # phase B behind ~9us of trailing DVE work).
        evt = s0.enter_context(tc.tile_pool(name="evt", bufs=1))
        with ExitStack() as sa:
            wp = sa.enter_context(tc.tile_pool(name="wts", bufs=1))
            wq_sb = wp.tile([128, KC * FPC], bf16, tag="wq")
            wk_sb = wp.tile([128, KC * DH], bf16, tag="wk")
            wv_sb = wp.tile([128, KC * DH], bf16, tag="wv")
            # rope tables in the never-released pool: the last n-tile's rope
            # reads them, and they must not pin the weight pool's release
            tabA = evt.tile([128, T], f32, tag="tabA")
            tabB = evt.tile([128, T], f32, tag="tabB")

            SLAB = 4
            NSLAB = KC // SLAB

            def load_w_slab(s):
                # one batched 3D-AP DMA per weight: DMA *issue* time on the
                # Sync queue (~0.6us each) is the startup bottleneck, not HBM.
                # Slab 0 splits out chunk 0 of Wq so the first matmul can
                # start as soon as ~0.25MB (vs ~2.3MB) has landed.
                k0, k1 = s * SLAB, (s + 1) * SLAB
                if s == 0:
                    nc.sync.dma_start(wq_sb[:, 0:FPC], wq[0:128, :])
                    nc.sync.dma_start(
                        wq_sb[:, FPC:k1 * FPC],
                        wq[128:k1 * 128, :].rearrange("(j p) c -> p j c", p=128))
                else:
                    nc.sync.dma_start(
                        wq_sb[:, k0 * FPC:k1 * FPC],
                        wq[k0 * 128:k1 * 128, :].rearrange("(j p) c -> p j c", p=128))
                nc.sync.dma_start(
                    wk_sb[:, k0 * DH:k1 * DH],
                    wk[k0 * 128:k1 * 128, :].rearrange("(j p) c -> p j c", p=128))
                nc.sync.dma_start(
                    wv_sb[:, k0 * DH:k1 * DH],
                    wv[k0 * 128:k1 * 128, :].rearrange("(j p) c -> p j c", p=128))

            xsp = sa.enter_context(tc.tile_pool(name="xs", bufs=3))
            evp = sa.enter_context(tc.tile_pool(name="ev", bufs=1))
            psA = sa.enter_context(tc.tile_pool(name="psA", bufs=1, space="PSUM"))

            def stationary(m, k):
                if m < HPC:
                    return wq_sb[:, k * FPC + m * 128: k * FPC + (m + 1) * 128]
                if m == HPC:
                    return wk_sb[:, k * DH:(k + 1) * DH]
                return wv_sb[:, k * DH:(k + 1) * DH]

            pending_vt = None

            def flush_vt():
                nonlocal pending_vt
                if pending_vt is None:
                    return
                vt_p, n_p = pending_vt
                b_p = n_p // 4
                for i in range(4):
                    ptr = psA.tile([128, 128], f32r, tag="tr", bufs=2, name="ptr")
                    nc.tensor.transpose(ptr[:], vt_p[:, i * 128:(i + 1) * 128], ident[:])
                    c_local = 4 * (n_p % 4) + i
                    nc.scalar.copy(Vb[b_p][:, c_local * 128:c_local * 128 + 128], ptr[:])
                pending_vt = None

            for n in range(NT // 512):
                b, tloc = n // 4, 512 * (n % 4)
                ps = [psA.tile([128, 512], f32, tag=f"ps{m}", name=f"ps{m}")
                      for m in range(6)]
                for s in range(NSLAB):
                    if n == 0:
                        load_w_slab(s)
                    xsl = xsp.tile([128, SLAB * 512], bf16, tag="xs", name="xsl")
                    if n == 0 and s == 0:
                        nc.sync.dma_start(xsl[:, 0:512], xT[0:128, 0:512])
                        nc.sync.dma_start(
                            xsl[:, 512:],
                            xT[128:SLAB * 128, 0:512].rearrange(
                                "(j p) c -> p j c", p=128))
                        # deferred setup loads: not needed until ~45us in
                        nc.sync.dma_start(tri_sb[:], trid)
                        nc.sync.dma_start(tabA[:], ropeA)
                        nc.sync.dma_start(tabB[:], ropeB)
                    else:
                        nc.sync.dma_start(
                            xsl[:],
                            xT[s * SLAB * 128:(s + 1) * SLAB * 128,
                               n * 512:(n + 1) * 512].rearrange(
                                   "(j p) c -> p j c", p=128))
                    for m in range(6):
                        for j in range(SLAB):
                            k = s * SLAB + j
                            nc.tensor.matmul(ps[m][:], stationary(m, k),
                                             xsl[:, j * 512:(j + 1) * 512],
                                             start=(k == 0), stop=(k == KC - 1))
                    if s == 0:
                        flush_vt()   # prev n-tile's V transposes, PE already warm
                # evict: ACT copies free PSUM banks at ACT pace; on the last
                # n-tile, flush V first and split evictions across ACT/DVE so
                # phase B's PSUM banks free as early as possible
                last = n == NT // 512 - 1
                pool = evt if last else evp
                qes = []
                if last:
                    vt = pool.tile([128, 512], f32r, tag="vt", bufs=1, name="vt")
                    nc.scalar.copy(vt[:], ps[5][:])
                    pending_vt = (vt, n)
                    flush_vt()
                    for m in range(5):
                        qe = pool.tile([128, 512], f32, tag="qe", bufs=5, name=f"qe{m}")
                        if m % 2 == 1:
                            nc.vector.tensor_copy(qe[:], ps[m][:])
                        else:
                            nc.scalar.copy(qe[:], ps[m][:])
                        qes.append(qe)
                else:
                    for m in range(5):
                        qe = pool.tile([128, 512], f32, tag="qe", bufs=6, name=f"qe{m}")
                        nc.scalar.copy(qe[:], ps[m][:])
                        qes.append(qe)
                    vt = pool.tile([128, 512], f32r, tag="vt", bufs=2, name="vt")
                    nc.scalar.copy(vt[:], ps[5][:])
                    pending_vt = (vt, n)
                # rope chains on DVE
                tA = tabA[:, tloc:tloc + 512]
                tB = tabB[:, tloc:tloc + 512]
                for m in range(5):
                    qe = qes[m]
                    sw = pool.tile([128, 512], f32, tag="sw", bufs=1, name="sw")
                    nc.vector.tensor_copy(sw[0:64, :], qe[64:128, :])
                    nc.vector.tensor_copy(sw[64:128, :], qe[0:64, :])
                    mm = pool.tile([128, 512], f32, tag="mm", bufs=1, name="mm")
                    nc.vector.tensor_mul(mm[:], sw[:], tB)
                    tt = pool.tile([128, 512], f32, tag="tt", bufs=1, name="tt")
                    nc.vector.tensor_mul(tt[:], qe[:], tA)
                    if m < HPC:
                        ro = pool.tile([128, 512], f32r, tag="ro", bufs=2, name="ro")
                        nc.vector.tensor_add(ro[:], tt[:], mm[:])
                        nc.sync.dma_start(qTd[m * 128:(m + 1) * 128, n * 512:(n + 1) * 512], ro[:])
                    else:
                        nc.vector.tensor_add(KTb[b][:, tloc:tloc + 512], tt[:], mm[:])
                if n == 3:
                    # qTd rows for head 0 of batch 0 are complete: prefetch
                    # the first attention Q tile while phase A continues
                    nc.sync.dma_start(qt0[:], qTd[0:128, 0:T])

        # ---------------- Phase B: attention (ctx -> SBUF bf16) --------
        # Pool-open order controls space reuse (stack allocator): the
        # attention-critical pools (sxp/smp/qtp) open first so they land in
        # the weight pool's space, whose release only waits on phase A's
        # matmuls. ctx/wo open after and overlap the eviction pool, whose
        # release waits on the final rope DMAs — but their first use is well
        # into phase B. All SBUF pools stay open until kernel end (no stack
        # pops mid-kernel); only psB releases before phase C's PSUM pool.
        sxp = s0.enter_context(tc.tile_pool(name="sxp", bufs=6))
        smp = s0.enter_context(tc.tile_pool(name="smp", bufs=2))
        qtp = s0.enter_context(tc.tile_pool(name="qtp", bufs=2))
        ctxp = s0.enter_context(tc.tile_pool(name="ctxp", bufs=1))
        ctx_sb = [ctxp.tile([128, NT], bf16, tag=f"ctx{h}", name=f"ctx{h}")
                  for h in range(HPC)]
        wop = s0.enter_context(tc.tile_pool(name="wop", bufs=1))
        wo_sb = wop.tile([128, HPC * D], bf16, tag="wo")

        # psB split: the normalize tiles (ctx accumulator + broadcast bank)
        # outlive the score/rowsum banks, so phase C's PSUM pool can allocate
        # as soon as the main attention banks release.
        psBt = s0.enter_context(tc.tile_pool(name="psBt", bufs=2, space="PSUM"))
        with ExitStack() as sbc:
            psB = sbc.enter_context(tc.tile_pool(name="psB", bufs=2, space="PSUM"))

            pending_norm = None

            def flush_norm():
                nonlocal pending_norm
                if pending_norm is None:
                    return
                ps_ctx_p, rs_p, h_p, b_p, qt_p = pending_norm
                # broadcast rs across partitions on the (otherwise idle)
                # GPSIMD engine -- no PE matmul, no DVE copy
                bcs = smp.tile([128, 512], f32, tag="bcs", name="bcs")
                nc.gpsimd.partition_broadcast(bcs[:], rs_p[:])
                nc.vector.tensor_mul(
                    ctx_sb[h_p][:, b_p * T + qt_p * 512: b_p * T + (qt_p + 1) * 512],
                    ps_ctx_p[:], bcs[:])
                pending_norm = None

            for b in range(B):
                for h in range(HPC):
                    if b == 0 and h == 0:
                        qt_t = qt0
                    else:
                        qt_t = qtp.tile([128, T], f32r, tag="qt")
                        nc.sync.dma_start(qt_t[:], qTd[h * 128:(h + 1) * 128, b * T:(b + 1) * T])
                    if b == 0:
                        # prefetch one Wo head-slab per head; space frees as
                        # phase A's weight pool closes
                        nc.sync.dma_start(wo_sb[:, h * D:(h + 1) * D],
                                          wo[h * 128:(h + 1) * 128, :])
                    for qt in range(4):
                        ps_ctx = psBt.tile([128, 512], f32, tag="ctx")
                        ps_sm = psB.tile([1, 512], f32, tag="sm")
                        nk = 4 * qt + 4

                        def issue_st(kt):
                            off = max(0, (kt - 4 * qt) * 128)
                            ps_st = psB.tile([128, 512], f32, tag="st", bufs=3, name="ps_st")
                            nc.tensor.matmul(ps_st[:, off:],
                                             KTb[b][:, kt * 128:(kt + 1) * 128],
                                             qt_t[:, qt * 512 + off:(qt + 1) * 512],
                                             start=True, stop=True)
                            se = sxp.tile([128, 512], f32r, tag="se", name="se")
                            nc.scalar.activation(se[:, off:], ps_st[:, off:],
                                                 mybir.ActivationFunctionType.Exp,
                                                 scale=EXP_SCALE)
                            if kt >= 4 * qt:
                                nc.vector.tensor_mul(se[:, off:off + 128],
                                                     se[:, off:off + 128], tri_sb[:])
                            return se, off

                        se_q = [issue_st(0), issue_st(1)]
                        for kt in range(nk):
                            se_cur, off = se_q.pop(0)
                            if kt + 2 < nk:
                                se_q.append(issue_st(kt + 2))
                            nc.tensor.matmul(ps_ctx[:, off:],
                                             Vb[b][:, kt * 128:(kt + 1) * 128],
                                             se_cur[:, off:],
                                             start=(kt == 0), stop=(kt == nk - 1))
                            nc.tensor.matmul(ps_sm[0:1, off:], ones[:], se_cur[:, off:],
                                             start=(kt == 0), stop=(kt == nk - 1))
                            if kt == 2:
                                flush_norm()  # prev q-tile's normalize, PE already busy
                        # reciprocal immediately (fast approx), consume one tile later
                        rs_f = smp.tile([1, 512], f32, tag="rsf", name="rs_f")
                        nc.vector.reciprocal_approx_fast(out=rs_f[:], in_=ps_sm[:])
                        pending_norm = (ps_ctx, rs_f, h, b, qt)

        # psB (score/rowsum banks) released here; the final q-tile's
        # normalize uses only psBt and runs concurrently with phase C's start
        flush_norm()

        # ---------------- Phase C: output projection (SBUF bf16) ----
        with ExitStack() as sc:
            psC = sc.enter_context(tc.tile_pool(name="psC", bufs=4, space="PSUM"))
            obp = sc.enter_context(tc.tile_pool(name="obp", bufs=2))
            for m in range(NT // 128):
                ob = obp.tile([128, D], bf16, tag="ob")
                for n in range(D // 512):
                    pso = psC.tile([128, 512], f32, tag="oc")
                    for h in range(HPC):
                        nc.tensor.matmul(pso[:],
                                         ctx_sb[h][:, m * 128:(m + 1) * 128],
                                         wo_sb[:, h * D + n * 512: h * D + (n + 1) * 512],
                                         start=(h == 0), stop=(h == HPC - 1))
                    # alternate eviction engines so neither stalls the PE
                    if n % 2 == 0:
                        nc.scalar.copy(ob[:, n * 512:(n + 1) * 512], pso[:])
                    else:
                        nc.vector.tensor_copy(ob[:, n * 512:(n + 1) * 512], pso[:])
                    if m == NT // 128 - 1 and n % 2 == 1:
                        # split the final tile's output DMA so the kernel tail
                        # isn't one serial 1MB transfer after the last eviction
                        nc.sync.dma_start(
                            outp[m * 128:(m + 1) * 128, (n - 1) * 512:(n + 1) * 512],
                            ob[:, (n - 1) * 512:(n + 1) * 512])
                if m < NT // 128 - 1:
                    nc.sync.dma_start(outp[m * 128:(m + 1) * 128, :], ob[:])

    nc.compile()
    return nc


def _get_nc():
    if "nc" not in _NC_CACHE:
        _NC_CACHE["nc"] = _build_program()
    return _NC_CACHE["nc"]


def _rope_tables():
    j = np.arange(0, DH, 2, dtype=np.float32) / np.float32(DH)
    inv_freq = (np.float32(1.0) / (np.float32(ROPE_BASE) ** j)).astype(np.float32)
    t = np.arange(T, dtype=np.float32)
    freqs = np.outer(t, inv_freq).astype(np.float32)   # (T, 64)
    c = np.cos(freqs).astype(np.float32).T             # (64, T)
    s = np.sin(freqs).astype(np.float32).T
    A = np.vstack([c, c]).astype(np.float32)           # (128, T)
    Bt = np.vstack([-s, s]).astype(np.float32)
    return np.ascontiguousarray(A), np.ascontiguousarray(Bt)


def _tri_mask():
    p = np.arange(128)[:, None]
    f = np.arange(128)[None, :]
    return np.ascontiguousarray((p <= f).astype(np.float32))


def _build_in_maps(x, Wq, Wk, Wv, Wo):
    import ml_dtypes

    bf = ml_dtypes.bfloat16
    xT = np.ascontiguousarray(x.reshape(NT, D).T).astype(bf)
    A, Bt = _rope_tables()
    tri = _tri_mask()
    in_maps = []
    for g in range(8):
        in_maps.append({
            "xT": xT,
            "wq": np.ascontiguousarray(Wq[:, g * FPC:(g + 1) * FPC]).astype(bf),
            "wk": np.ascontiguousarray(Wk[:, g * DH:(g + 1) * DH]).astype(bf),
            "wv": np.ascontiguousarray(Wv[:, g * DH:(g + 1) * DH]).astype(bf),
            "wo": np.ascontiguousarray(
                Wo[g * FPC:(g + 1) * FPC, :]).astype(bf),
            "ropeA": A,
            "ropeB": Bt,
            "trid": _round_fp32r(tri),
        })
    return in_maps


def kernel(x, Wq, Wk, Wv, Wo):
    x = np.asarray(x, dtype=np.float32)
    Wq = np.asarray(Wq, dtype=np.float32)
    Wk = np.asarray(Wk, dtype=np.float32)
    Wv = np.asarray(Wv, dtype=np.float32)
    Wo = np.asarray(Wo, dtype=np.float32)

    nc = _get_nc()
    in_maps = _build_in_maps(x, Wq, Wk, Wv, Wo)

    res = run_bass_kernel_spmd(nc, in_maps, list(range(8)))
    acc = res.results[0]["outp"].astype(np.float32)
    for g in range(1, 8):
        acc = acc + res.results[g]["outp"].astype(np.float32)
    return np.ascontiguousarray(acc.reshape(B, T, D), dtype=np.float32)


# revision 33
# speedup vs baseline: 1.0157x; 1.0066x over previous
"""GroupedQueryAttention TRN2 Bass kernel, sharded over 8 NeuronCores.

Problem (hardcoded): B=2, T=2048, D=4096, 32 Q heads x 128, 8 KV groups x 128,
RoPE (base 5e5), causal, out = ctx @ Wo.

Sharding: core g owns Q heads 4g..4g+3 (Wq columns 512g:512g+512), KV group g
(Wk/Wv columns 128g:128g+128), and Wo rows 512g:512g+512 (row-parallel).
Each core computes a full-shape partial output (bf16); host sums the 8
partials in fp32.

Matmuls run in float32r (fp32 with 11-bit mantissa, full PE rate); the
attention context and output projection run in bf16 (same PE rate, half the
SBUF/DMA traffic). Inputs are pre-rounded to fp32r on host.

Versus the 1.14ms baseline (final ~0.82ms, all phases 97-99% PE-busy):
 - softmax reciprocal via reciprocal_approx_fast (DVE reciprocal is an
   iterative divide, 3.3us per q-tile, and stalled the PE every tile);
   the denominator broadcast runs on the idle GPSIMD engine
 - causal diagonal blocks narrowed: score/exp/AV/rowsum matmuls only cover
   valid query columns (N = 512-128r); single [128,128] triangle mask
 - x/Wq/Wk/Wv in bf16 (same PE rate as f32r, half the DMA); attention
   GEMMs stay f32r (bf16 moving operands measured slower there)
 - small DMAs batched into 3D-AP transfers: the Sync queue's ~0.6us
   per-issue cost, not HBM bandwidth, limited the startup
 - ctx kept in SBUF as bf16 (no DRAM round-trip), Wo in bf16, output
   partials written bf16; one 1MB output DMA per token tile
 - pool scoping tuned so no phase waits on a prior phase's stragglers
   (eviction/rope tiles of the last tile live in a never-released pool)
 - exp ACT table preloaded at kernel start; Wo and the first Q tile
   prefetched during earlier phases
"""
import sys
import numpy as np

for _p in ("/opt/trn_rl_repo", "/root/.axon_site", "/root/.axon_site/_ro/trn_rl_repo"):
    if _p not in sys.path:
        sys.path.append(_p)

from contextlib import ExitStack

import concourse.bass as bass
import concourse.tile as tile
from concourse import bacc, mybir
from concourse.bass_utils import run_bass_kernel_spmd
from concourse.masks import make_identity

B, T, D = 2, 2048, 4096
NH, NKV, DH = 32, 8, 128
HPC = NH // 8          # 4 q heads per core
FPC = HPC * DH         # 512 q features per core
ROPE_BASE = 500000.0
NT = B * T             # 4096 tokens
f32 = mybir.dt.float32
f32r = mybir.dt.float32r
bf16 = mybir.dt.bfloat16
EXP_SCALE = 1.0 / float(np.sqrt(DH))

_NC_CACHE = {}


def _round_fp32r(x):
    x = np.ascontiguousarray(x, dtype=np.float32)
    u = x.view(np.uint32)
    lsb = (u >> 12) & np.uint32(1)
    r = (u + np.uint32(0x7FF) + lsb) & np.uint32(0xFFFFF000)
    return r.view(np.float32)


def _build_program():
    nc = bacc.Bacc("TRN2", target_bir_lowering=False, debug=False)

    xT = nc.dram_tensor("xT", [D, NT], bf16, kind="ExternalInput").ap()
    wq = nc.dram_tensor("wq", [D, FPC], bf16, kind="ExternalInput").ap()
    wk = nc.dram_tensor("wk", [D, DH], bf16, kind="ExternalInput").ap()
    wv = nc.dram_tensor("wv", [D, DH], bf16, kind="ExternalInput").ap()
    wo = nc.dram_tensor("wo", [FPC, D], bf16, kind="ExternalInput").ap()
    ropeA = nc.dram_tensor("ropeA", [128, T], f32, kind="ExternalInput").ap()
    ropeB = nc.dram_tensor("ropeB", [128, T], f32, kind="ExternalInput").ap()
    trid = nc.dram_tensor("trid", [128, 128], f32r, kind="ExternalInput").ap()
    outp = nc.dram_tensor("outp", [NT, D], bf16, kind="ExternalOutput").ap()

    qTd = nc.dram_tensor("qTd", [FPC, NT], f32r).ap()     # roped Q^T, feature-major

    KC = D // 128  # 32 contraction chunks

    with tile.TileContext(nc) as tc, ExitStack() as s0:
        kvp = s0.enter_context(tc.tile_pool(name="kv", bufs=1))
        KTb = [kvp.tile([128, T], f32r, tag=f"KT{i}", name=f"KT{i}") for i in range(B)]
        Vb = [kvp.tile([128, T], f32r, tag=f"V{i}", name=f"V{i}") for i in range(B)]
        ident_f = kvp.tile([128, 128], f32, tag="ident_f")
        make_identity(nc, ident_f[:])
        ident = kvp.tile([128, 128], f32r, tag="ident")
        nc.vector.tensor_copy(ident[:], ident_f[:])
        ones_f = kvp.tile([128, 1], f32, tag="ones_f")
        nc.vector.memset(ones_f[:], 1.0)
        ones = kvp.tile([128, 1], f32r, tag="ones")
        nc.vector.tensor_copy(ones[:], ones_f[:])
        ones_row_f = kvp.tile([1, 128], f32, tag="ones_row_f")
        nc.vector.memset(ones_row_f[:], 1.0)
        ones_row = kvp.tile([1, 128], f32r, tag="ones_row")
        nc.vector.tensor_copy(ones_row[:], ones_row_f[:])
        tri_sb = kvp.tile([128, 128], f32r, tag="tri")
        # first Q-head tile, prefetched during phase A (n==3) so attention
        # starts without a DMA stall
        qt0 = kvp.tile([128, T], f32r, tag="qt0")
        # preload the exp table set so the first attention exp doesn't pay
        # the ~2.7us ACT_TABLE_LOAD mid-kernel
        warm = kvp.tile([1, 2], f32, tag="warm")
        nc.vector.memset(warm[:], 0.0)
        nc.scalar.activation(warm[0:1, 0:1], warm[0:1, 1:2],
                             mybir.ActivationFunctionType.Exp, scale=1.0)

        # ---------------- Phase A: projections + RoPE -----------------
        # Eviction/rope tiles for the LAST n-tile live in a pool that stays
        # open to kernel end: phase B's pools then never wait on the final
        # rope chain (the A-scoped pools' releases would otherwise serialize
        # phase B behind ~9us of trailing DVE work).
        evt = s0.enter_context(tc.tile_pool(name="evt", bufs=1))
        with ExitStack() as sa:
            wp = sa.enter_context(tc.tile_pool(name="wts", bufs=1))
            wq_sb = wp.tile([128, KC * FPC], bf16, tag="wq")
            wk_sb = wp.tile([128, KC * DH], bf16, tag="wk")
            wv_sb = wp.tile([128, KC * DH], bf16, tag="wv")
            # rope tables in the never-released pool: the last n-tile's rope
            # reads them, and they must not pin the weight pool's release
            tabA = evt.tile([128, T], f32, tag="tabA")
            tabB = evt.tile([128, T], f32, tag="tabB")

            SLAB = 4
            NSLAB = KC // SLAB

            def load_w_slab(s):
                # one batched 3D-AP DMA per weight: DMA *issue* time on the
                # Sync queue (~0.6us each) is the startup bottleneck, not HBM.
                # Slab 0 splits out chunk 0 of Wq so the first matmul can
                # start as soon as ~0.25MB (vs ~2.3MB) has landed.
                k0, k1 = s * SLAB, (s + 1) * SLAB
                nc.sync.dma_start(
                    wq_sb[:, k0 * FPC:k1 * FPC],
                    wq[k0 * 128:k1 * 128, :].rearrange("(j p) c -> p j c", p=128))
                nc.sync.dma_start(
                    wk_sb[:, k0 * DH:k1 * DH],
                    wk[k0 * 128:k1 * 128, :].rearrange("(j p) c -> p j c", p=128))
                nc.sync.dma_start(
                    wv_sb[:, k0 * DH:k1 * DH],
                    wv[k0 * 128:k1 * 128, :].rearrange("(j p) c -> p j c", p=128))

            xsp = sa.enter_context(tc.tile_pool(name="xs", bufs=3))
            evp = sa.enter_context(tc.tile_pool(name="ev", bufs=1))
            psA = sa.enter_context(tc.tile_pool(name="psA", bufs=1, space="PSUM"))

            def stationary(m, k):
                if m < HPC:
                    return wq_sb[:, k * FPC + m * 128: k * FPC + (m + 1) * 128]
                if m == HPC:
                    return wk_sb[:, k * DH:(k + 1) * DH]
                return wv_sb[:, k * DH:(k + 1) * DH]

            pending_vt = None

            def flush_vt():
                nonlocal pending_vt
                if pending_vt is None:
                    return
                vt_p, n_p = pending_vt
                b_p = n_p // 4
                for i in range(4):
                    ptr = psA.tile([128, 128], f32r, tag="tr", bufs=2, name="ptr")
                    nc.tensor.transpose(ptr[:], vt_p[:, i * 128:(i + 1) * 128], ident[:])
                    c_local = 4 * (n_p % 4) + i
                    nc.scalar.copy(Vb[b_p][:, c_local * 128:c_local * 128 + 128], ptr[:])
                pending_vt = None

            for n in range(NT // 512):
                b, tloc = n // 4, 512 * (n % 4)
                ps = [psA.tile([128, 512], f32, tag=f"ps{m}", name=f"ps{m}")
                      for m in range(6)]
                for s in range(NSLAB):
                    if n == 0:
                        load_w_slab(s)
                    xsl = xsp.tile([128, SLAB * 512], bf16, tag="xs", name="xsl")
                    nc.sync.dma_start(
                        xsl[:],
                        xT[s * SLAB * 128:(s + 1) * SLAB * 128,
                           n * 512:(n + 1) * 512].rearrange(
                               "(j p) c -> p j c", p=128))
                    if n == 0 and s == 0:
                        # deferred setup loads: not needed until ~45us in
                        nc.sync.dma_start(tri_sb[:], trid)
                        nc.sync.dma_start(tabA[:], ropeA)
                        nc.sync.dma_start(tabB[:], ropeB)
                    for m in range(6):
                        for j in range(SLAB):
                            k = s * SLAB + j
                            nc.tensor.matmul(ps[m][:], stationary(m, k),
                                             xsl[:, j * 512:(j + 1) * 512],
                                             start=(k == 0), stop=(k == KC - 1))
                    if s == 0:
                        flush_vt()   # prev n-tile's V transposes, PE already warm
                # evict: ACT copies free PSUM banks at ACT pace; on the last
                # n-tile, flush V first and split evictions across ACT/DVE so
                # phase B's PSUM banks free as early as possible
                last = n == NT // 512 - 1
                pool = evt if last else evp
                qes = []
                if last:
                    vt = pool.tile([128, 512], f32r, tag="vt", bufs=1, name="vt")
                    nc.scalar.copy(vt[:], ps[5][:])
                    pending_vt = (vt, n)
                    flush_vt()
                    for m in range(5):
                        qe = pool.tile([128, 512], f32, tag="qe", bufs=5, name=f"qe{m}")
                        if m % 2 == 1:
                            nc.vector.tensor_copy(qe[:], ps[m][:])
                        else:
                            nc.scalar.copy(qe[:], ps[m][:])
                        qes.append(qe)
                else:
                    for m in range(5):
                        qe = pool.tile([128, 512], f32, tag="qe", bufs=6, name=f"qe{m}")
                        nc.scalar.copy(qe[:], ps[m][:])
                        qes.append(qe)
                    vt = pool.tile([128, 512], f32r, tag="vt", bufs=2, name="vt")
                    nc.scalar.copy(vt[:], ps[5][:])
                    pending_vt = (vt, n)
                # rope chains on DVE
                tA = tabA[:, tloc:tloc + 512]
                tB = tabB[:, tloc:tloc + 512]
                for m in range(5):
                    qe = qes[m]
                    sw = pool.tile([128, 512], f32, tag="sw", bufs=1, name="sw")
                    nc.vector.tensor_copy(sw[0:64, :], qe[64:128, :])
                    nc.vector.tensor_copy(sw[64:128, :], qe[0:64, :])
                    mm = pool.tile([128, 512], f32, tag="mm", bufs=1, name="mm")
                    nc.vector.tensor_mul(mm[:], sw[:], tB)
                    tt = pool.tile([128, 512], f32, tag="tt", bufs=1, name="tt")
                    nc.vector.tensor_mul(tt[:], qe[:], tA)
                    if m < HPC:
                        ro = pool.tile([128, 512], f32r, tag="ro", bufs=2, name="ro")
                        nc.vector.tensor_add(ro[:], tt[:], mm[:])
                        nc.sync.dma_start(qTd[m * 128:(m + 1) * 128, n * 512:(n + 1) * 512], ro[:])
                    else:
                        nc.vector.tensor_add(KTb[b][:, tloc:tloc + 512], tt[:], mm[:])
                if n == 3:
                    # qTd rows for head 0 of batch 0 are complete: prefetch
                    # the first attention Q tile while phase A continues
                    nc.sync.dma_start(qt0[:], qTd[0:128, 0:T])

        # ---------------- Phase B: attention (ctx -> SBUF bf16) --------
        # Pool-open order controls space reuse (stack allocator): the
        # attention-critical pools (sxp/smp/qtp) open first so they land in
        # the weight pool's space, whose release only waits on phase A's
        # matmuls. ctx/wo open after and overlap the eviction pool, whose
        # release waits on the final rope DMAs — but their first use is well
        # into phase B. All SBUF pools stay open until kernel end (no stack
        # pops mid-kernel); only psB releases before phase C's PSUM pool.
        sxp = s0.enter_context(tc.tile_pool(name="sxp", bufs=6))
        smp = s0.enter_context(tc.tile_pool(name="smp", bufs=2))
        qtp = s0.enter_context(tc.tile_pool(name="qtp", bufs=2))
        ctxp = s0.enter_context(tc.tile_pool(name="ctxp", bufs=1))
        ctx_sb = [ctxp.tile([128, NT], bf16, tag=f"ctx{h}", name=f"ctx{h}")
                  for h in range(HPC)]
        wop = s0.enter_context(tc.tile_pool(name="wop", bufs=1))
        wo_sb = wop.tile([128, HPC * D], bf16, tag="wo")

        # psB split: the normalize tiles (ctx accumulator + broadcast bank)
        # outlive the score/rowsum banks, so phase C's PSUM pool can allocate
        # as soon as the main attention banks release.
        psBt = s0.enter_context(tc.tile_pool(name="psBt", bufs=2, space="PSUM"))
        with ExitStack() as sbc:
            psB = sbc.enter_context(tc.tile_pool(name="psB", bufs=2, space="PSUM"))

            pending_norm = None

            def flush_norm():
                nonlocal pending_norm
                if pending_norm is None:
                    return
                ps_ctx_p, rs_p, h_p, b_p, qt_p = pending_norm
                # broadcast rs across partitions on the (otherwise idle)
                # GPSIMD engine -- no PE matmul, no DVE copy
                bcs = smp.tile([128, 512], f32, tag="bcs", name="bcs")
                nc.gpsimd.partition_broadcast(bcs[:], rs_p[:])
                nc.vector.tensor_mul(
                    ctx_sb[h_p][:, b_p * T + qt_p * 512: b_p * T + (qt_p + 1) * 512],
                    ps_ctx_p[:], bcs[:])
                pending_norm = None

            for b in range(B):
                for h in range(HPC):
                    if b == 0 and h == 0:
                        qt_t = qt0
                    else:
                        qt_t = qtp.tile([128, T], f32r, tag="qt")
                        nc.sync.dma_start(qt_t[:], qTd[h * 128:(h + 1) * 128, b * T:(b + 1) * T])
                    if b == 1:
                        # prefetch one Wo head-slab per head during the second
                        # batch; issuing at b==0 would block the in-order Sync
                        # queue behind the eviction pool's release and delay
                        # the next head's Q-tile load
                        nc.sync.dma_start(wo_sb[:, h * D:(h + 1) * D],
                                          wo[h * 128:(h + 1) * 128, :])
                    for qt in range(4):
                        ps_ctx = psBt.tile([128, 512], f32, tag="ctx")
                        ps_sm = psB.tile([1, 512], f32, tag="sm")
                        nk = 4 * qt + 4

                        def issue_st(kt):
                            off = max(0, (kt - 4 * qt) * 128)
                            ps_st = psB.tile([128, 512], f32, tag="st", bufs=3, name="ps_st")
                            nc.tensor.matmul(ps_st[:, off:],
                                             KTb[b][:, kt * 128:(kt + 1) * 128],
                                             qt_t[:, qt * 512 + off:(qt + 1) * 512],
                                             start=True, stop=True)
                            se = sxp.tile([128, 512], f32r, tag="se", name="se")
                            nc.scalar.activation(se[:, off:], ps_st[:, off:],
                                                 mybir.ActivationFunctionType.Exp,
                                                 scale=EXP_SCALE)
                            if kt >= 4 * qt:
                                nc.vector.tensor_mul(se[:, off:off + 128],
                                                     se[:, off:off + 128], tri_sb[:])
                            return se, off

                        se_q = [issue_st(0), issue_st(1)]
                        for kt in range(nk):
                            se_cur, off = se_q.pop(0)
                            if kt + 2 < nk:
                                se_q.append(issue_st(kt + 2))
                            nc.tensor.matmul(ps_ctx[:, off:],
                                             Vb[b][:, kt * 128:(kt + 1) * 128],
                                             se_cur[:, off:],
                                             start=(kt == 0), stop=(kt == nk - 1))
                            nc.tensor.matmul(ps_sm[0:1, off:], ones[:], se_cur[:, off:],
                                             start=(kt == 0), stop=(kt == nk - 1))
                            if kt == 2:
                                flush_norm()  # prev q-tile's normalize, PE already busy
                        # reciprocal immediately (fast approx), consume one tile later
                        rs_f = smp.tile([1, 512], f32, tag="rsf", name="rs_f")
                        nc.vector.reciprocal_approx_fast(out=rs_f[:], in_=ps_sm[:])
                        pending_norm = (ps_ctx, rs_f, h, b, qt)

        # psB (score/rowsum banks) released here; the final q-tile's
        # normalize uses only psBt and runs concurrently with phase C's start
        flush_norm()

        # ---------------- Phase C: output projection (SBUF bf16) ----
        with ExitStack() as sc:
            psC = sc.enter_context(tc.tile_pool(name="psC", bufs=4, space="PSUM"))
            obp = sc.enter_context(tc.tile_pool(name="obp", bufs=2))
            for m in range(NT // 128):
                ob = obp.tile([128, D], bf16, tag="ob")
                for n in range(D // 512):
                    pso = psC.tile([128, 512], f32, tag="oc")
                    for h in range(HPC):
                        nc.tensor.matmul(pso[:],
                                         ctx_sb[h][:, m * 128:(m + 1) * 128],
                                         wo_sb[:, h * D + n * 512: h * D + (n + 1) * 512],
                                         start=(h == 0), stop=(h == HPC - 1))
                    # alternate eviction engines so neither stalls the PE
                    if n % 2 == 0:
                        nc.scalar.copy(ob[:, n * 512:(n + 1) * 512], pso[:])
                    else:
                        nc.vector.tensor_copy(ob[:, n * 512:(n + 1) * 512], pso[:])
                    if m == NT // 128 - 1 and n % 2 == 1:
                        # split the final tile's output DMA so the kernel tail
                        # isn't one serial 1MB transfer after the last eviction
                        nc.sync.dma_start(
                            outp[m * 128:(m + 1) * 128, (n - 1) * 512:(n + 1) * 512],
                            ob[:, (n - 1) * 512:(n + 1) * 512])
                if m < NT // 128 - 1:
                    nc.sync.dma_start(outp[m * 128:(m + 1) * 128, :], ob[:])

    nc.compile()
    return nc


def _get_nc():
    if "nc" not in _NC_CACHE:
        _NC_CACHE["nc"] = _build_program()
    return _NC_CACHE["nc"]


def _rope_tables():
    j = np.arange(0, DH, 2, dtype=np.float32) / np.float32(DH)
    inv_freq = (np.float32(1.0) / (np.float32(ROPE_BASE) ** j)).astype(np.float32)
    t = np.arange(T, dtype=np.float32)
    freqs = np.outer(t, inv_freq).astype(np.float32)   # (T, 64)
    c = np.cos(freqs).astype(np.float32).T             # (64, T)
    s = np.sin(freqs).astype(np.float32).T
    A = np.vstack([c, c]).astype(np.float32)           # (128, T)
    Bt = np.vstack([-s, s]).astype(np.float32)
    return np.ascontiguousarray(A), np.ascontiguousarray(Bt)


def _tri_mask():
    p = np.arange(128)[:, None]
    f = np.arange(128)[None, :]
    return np.ascontiguousarray((p <= f).astype(np.float32))


def _build_in_maps(x, Wq, Wk, Wv, Wo):
    import ml_dtypes

    bf = ml_dtypes.bfloat16
    xT = np.ascontiguousarray(x.reshape(NT, D).T).astype(bf)
    A, Bt = _rope_tables()
    tri = _tri_mask()
    in_maps = []
    for g in range(8):
        in_maps.append({
            "xT": xT,
            "wq": np.ascontiguousarray(Wq[:, g * FPC:(g + 1) * FPC]).astype(bf),
            "wk": np.ascontiguousarray(Wk[:, g * DH:(g + 1) * DH]).astype(bf),
            "wv": np.ascontiguousarray(Wv[:, g * DH:(g + 1) * DH]).astype(bf),
            "wo": np.ascontiguousarray(
                Wo[g * FPC:(g + 1) * FPC, :]).astype(bf),
            "ropeA": A,
            "ropeB": Bt,
            "trid": _round_fp32r(tri),
        })
    return in_maps


def kernel(x, Wq, Wk, Wv, Wo):
    x = np.asarray(x, dtype=np.float32)
    Wq = np.asarray(Wq, dtype=np.float32)
    Wk = np.asarray(Wk, dtype=np.float32)
    Wv = np.asarray(Wv, dtype=np.float32)
    Wo = np.asarray(Wo, dtype=np.float32)

    nc = _get_nc()
    in_maps = _build_in_maps(x, Wq, Wk, Wv, Wo)

    res = run_bass_kernel_spmd(nc, in_maps, list(range(8)))
    acc = res.results[0]["outp"].astype(np.float32)
    for g in range(1, 8):
        acc = acc + res.results[g]["outp"].astype(np.float32)
    return np.ascontiguousarray(acc.reshape(B, T, D), dtype=np.float32)


# revision 35
# speedup vs baseline: 1.0275x; 1.0116x over previous
"""GroupedQueryAttention TRN2 Bass kernel, sharded over 8 NeuronCores.

Problem (hardcoded): B=2, T=2048, D=4096, 32 Q heads x 128, 8 KV groups x 128,
RoPE (base 5e5), causal, out = ctx @ Wo.

Sharding: core g owns Q heads 4g..4g+3 (Wq columns 512g:512g+512), KV group g
(Wk/Wv columns 128g:128g+128), and Wo rows 512g:512g+512 (row-parallel).
Each core computes a full-shape partial output (bf16); host sums the 8
partials in fp32.

Matmuls run in float32r (fp32 with 11-bit mantissa, full PE rate); the
attention context and output projection run in bf16 (same PE rate, half the
SBUF/DMA traffic). Inputs are pre-rounded to fp32r on host.

Versus the 1.14ms baseline (final ~0.82ms, all phases 97-99% PE-busy):
 - softmax reciprocal via reciprocal_approx_fast (DVE reciprocal is an
   iterative divide, 3.3us per q-tile, and stalled the PE every tile);
   the denominator broadcast runs on the idle GPSIMD engine
 - causal diagonal blocks narrowed: score/exp/AV/rowsum matmuls only cover
   valid query columns (N = 512-128r); single [128,128] triangle mask
 - x/Wq/Wk/Wv in bf16 (same PE rate as f32r, half the DMA); attention
   GEMMs stay f32r (bf16 moving operands measured slower there)
 - small DMAs batched into 3D-AP transfers: the Sync queue's ~0.6us
   per-issue cost, not HBM bandwidth, limited the startup
 - ctx kept in SBUF as bf16 (no DRAM round-trip), Wo in bf16, output
   partials written bf16; one 1MB output DMA per token tile
 - pool scoping tuned so no phase waits on a prior phase's stragglers
   (eviction/rope tiles of the last tile live in a never-released pool)
 - exp ACT table preloaded at kernel start; Wo and the first Q tile
   prefetched during earlier phases
"""
import sys
import numpy as np

for _p in ("/opt/trn_rl_repo", "/root/.axon_site", "/root/.axon_site/_ro/trn_rl_repo"):
    if _p not in sys.path:
        sys.path.append(_p)

from contextlib import ExitStack

import concourse.bass as bass
import concourse.tile as tile
from concourse import bacc, mybir
from concourse.bass_utils import run_bass_kernel_spmd
from concourse.masks import make_identity

B, T, D = 2, 2048, 4096
NH, NKV, DH = 32, 8, 128
HPC = NH // 8          # 4 q heads per core
FPC = HPC * DH         # 512 q features per core
ROPE_BASE = 500000.0
NT = B * T             # 4096 tokens
f32 = mybir.dt.float32
f32r = mybir.dt.float32r
bf16 = mybir.dt.bfloat16
EXP_SCALE = 1.0 / float(np.sqrt(DH))

_NC_CACHE = {}


def _round_fp32r(x):
    x = np.ascontiguousarray(x, dtype=np.float32)
    u = x.view(np.uint32)
    lsb = (u >> 12) & np.uint32(1)
    r = (u + np.uint32(0x7FF) + lsb) & np.uint32(0xFFFFF000)
    return r.view(np.float32)


def _build_program():
    nc = bacc.Bacc("TRN2", target_bir_lowering=False, debug=False)

    xT = nc.dram_tensor("xT", [D, NT], bf16, kind="ExternalInput").ap()
    wq = nc.dram_tensor("wq", [D, FPC], bf16, kind="ExternalInput").ap()
    wk = nc.dram_tensor("wk", [D, DH], bf16, kind="ExternalInput").ap()
    wv = nc.dram_tensor("wv", [D, DH], bf16, kind="ExternalInput").ap()
    wo = nc.dram_tensor("wo", [FPC, D], bf16, kind="ExternalInput").ap()
    ropeA = nc.dram_tensor("ropeA", [128, T], f32, kind="ExternalInput").ap()
    ropeB = nc.dram_tensor("ropeB", [128, T], f32, kind="ExternalInput").ap()
    trid = nc.dram_tensor("trid", [128, 128], f32r, kind="ExternalInput").ap()
    outp = nc.dram_tensor("outp", [NT, D], bf16, kind="ExternalOutput").ap()

    qTd = nc.dram_tensor("qTd", [FPC, NT], f32r).ap()     # roped Q^T, feature-major

    KC = D // 128  # 32 contraction chunks

    with tile.TileContext(nc) as tc, ExitStack() as s0:
        kvp = s0.enter_context(tc.tile_pool(name="kv", bufs=1))
        KTb = [kvp.tile([128, T], f32r, tag=f"KT{i}", name=f"KT{i}") for i in range(B)]
        Vb = [kvp.tile([128, T], f32r, tag=f"V{i}", name=f"V{i}") for i in range(B)]
        ident_f = kvp.tile([128, 128], f32, tag="ident_f")
        make_identity(nc, ident_f[:])
        ident = kvp.tile([128, 128], f32r, tag="ident")
        nc.vector.tensor_copy(ident[:], ident_f[:])
        ones_f = kvp.tile([128, 1], f32, tag="ones_f")
        nc.vector.memset(ones_f[:], 1.0)
        ones = kvp.tile([128, 1], f32r, tag="ones")
        nc.vector.tensor_copy(ones[:], ones_f[:])
        ones_row_f = kvp.tile([1, 128], f32, tag="ones_row_f")
        nc.vector.memset(ones_row_f[:], 1.0)
        ones_row = kvp.tile([1, 128], f32r, tag="ones_row")
        nc.vector.tensor_copy(ones_row[:], ones_row_f[:])
        tri_sb = kvp.tile([128, 128], f32r, tag="tri")
        # first Q-head tile, prefetched during phase A (n==3) so attention
        # starts without a DMA stall
        qt0 = kvp.tile([128, T], f32r, tag="qt0")
        # preload the exp table set so the first attention exp doesn't pay
        # the ~2.7us ACT_TABLE_LOAD mid-kernel
        warm = kvp.tile([1, 2], f32, tag="warm")
        nc.vector.memset(warm[:], 0.0)
        nc.scalar.activation(warm[0:1, 0:1], warm[0:1, 1:2],
                             mybir.ActivationFunctionType.Exp, scale=1.0)
        # warm up the GPSIMD ext-isa library too: the first
        # partition_broadcast otherwise pays a ~6us IRAM load mid-attention
        wbc = kvp.tile([128, 2], f32, tag="wbc")
        nc.gpsimd.partition_broadcast(wbc[:], warm[0:1, :])

        # ---------------- Phase A: projections + RoPE -----------------
        # Eviction/rope tiles for the LAST n-tile live in a pool that stays
        # open to kernel end: phase B's pools then never wait on the final
        # rope chain (the A-scoped pools' releases would otherwise serialize
        # phase B behind ~9us of trailing DVE work).
        evt = s0.enter_context(tc.tile_pool(name="evt", bufs=1))
        with ExitStack() as sa:
            wp = sa.enter_context(tc.tile_pool(name="wts", bufs=1))
            wq_sb = wp.tile([128, KC * FPC], bf16, tag="wq")
            wk_sb = wp.tile([128, KC * DH], bf16, tag="wk")
            wv_sb = wp.tile([128, KC * DH], bf16, tag="wv")
            # rope tables in the never-released pool: the last n-tile's rope
            # reads them, and they must not pin the weight pool's release
            tabA = evt.tile([128, T], f32, tag="tabA")
            tabB = evt.tile([128, T], f32, tag="tabB")

            SLAB = 4
            NSLAB = KC // SLAB

            def load_w_slab(s):
                # one batched 3D-AP DMA per weight: DMA *issue* time on the
                # Sync queue (~0.6us each) is the startup bottleneck, not HBM
                k0, k1 = s * SLAB, (s + 1) * SLAB
                nc.sync.dma_start(
                    wq_sb[:, k0 * FPC:k1 * FPC],
                    wq[k0 * 128:k1 * 128, :].rearrange("(j p) c -> p j c", p=128))
                nc.sync.dma_start(
                    wk_sb[:, k0 * DH:k1 * DH],
                    wk[k0 * 128:k1 * 128, :].rearrange("(j p) c -> p j c", p=128))
                nc.sync.dma_start(
                    wv_sb[:, k0 * DH:k1 * DH],
                    wv[k0 * 128:k1 * 128, :].rearrange("(j p) c -> p j c", p=128))

            xsp = sa.enter_context(tc.tile_pool(name="xs", bufs=4))
            evp = sa.enter_context(tc.tile_pool(name="ev", bufs=1))
            psA = sa.enter_context(tc.tile_pool(name="psA", bufs=1, space="PSUM"))

            def stationary(m, k):
                if m < HPC:
                    return wq_sb[:, k * FPC + m * 128: k * FPC + (m + 1) * 128]
                if m == HPC:
                    return wk_sb[:, k * DH:(k + 1) * DH]
                return wv_sb[:, k * DH:(k + 1) * DH]

            pending_vt = None

            def flush_vt():
                nonlocal pending_vt
                if pending_vt is None:
                    return
                vt_p, n_p = pending_vt
                b_p = n_p // 4
                for i in range(4):
                    ptr = psA.tile([128, 128], f32r, tag="tr", bufs=2, name="ptr")
                    nc.tensor.transpose(ptr[:], vt_p[:, i * 128:(i + 1) * 128], ident[:])
                    c_local = 4 * (n_p % 4) + i
                    nc.scalar.copy(Vb[b_p][:, c_local * 128:c_local * 128 + 128], ptr[:])
                pending_vt = None

            for n in range(NT // 512):
                b, tloc = n // 4, 512 * (n % 4)
                ps = [psA.tile([128, 512], f32, tag=f"ps{m}", name=f"ps{m}")
                      for m in range(6)]
                for s in range(NSLAB):
                    if n == 0:
                        load_w_slab(s)
                    xsl = xsp.tile([128, SLAB * 512], bf16, tag="xs", name="xsl")
                    nc.sync.dma_start(
                        xsl[:],
                        xT[s * SLAB * 128:(s + 1) * SLAB * 128,
                           n * 512:(n + 1) * 512].rearrange(
                               "(j p) c -> p j c", p=128))
                    if n == 0 and s == 0:
                        # deferred setup loads: not needed until ~45us in
                        nc.sync.dma_start(tri_sb[:], trid)
                        nc.sync.dma_start(tabA[:], ropeA)
                        nc.sync.dma_start(tabB[:], ropeB)
                    for m in range(6):
                        for j in range(SLAB):
                            k = s * SLAB + j
                            nc.tensor.matmul(ps[m][:], stationary(m, k),
                                             xsl[:, j * 512:(j + 1) * 512],
                                             start=(k == 0), stop=(k == KC - 1))
                    if s == 0:
                        flush_vt()   # prev n-tile's V transposes, PE already warm
                # evict: ACT copies free PSUM banks at ACT pace; on the last
                # n-tile, flush V first and split evictions across ACT/DVE so
                # phase B's PSUM banks free as early as possible
                last = n == NT // 512 - 1
                pool = evt if last else evp
                qes = []
                if last:
                    vt = pool.tile([128, 512], f32r, tag="vt", bufs=1, name="vt")
                    nc.scalar.copy(vt[:], ps[5][:])
                    pending_vt = (vt, n)
                    flush_vt()
                    for m in range(5):
                        qe = pool.tile([128, 512], f32, tag="qe", bufs=5, name=f"qe{m}")
                        if m % 2 == 1:
                            nc.vector.tensor_copy(qe[:], ps[m][:])
                        else:
                            nc.scalar.copy(qe[:], ps[m][:])
                        qes.append(qe)
                else:
                    for m in range(5):
                        qe = pool.tile([128, 512], f32, tag="qe", bufs=6, name=f"qe{m}")
                        nc.scalar.copy(qe[:], ps[m][:])
                        qes.append(qe)
                    vt = pool.tile([128, 512], f32r, tag="vt", bufs=2, name="vt")
                    nc.scalar.copy(vt[:], ps[5][:])
                    pending_vt = (vt, n)
                # rope chains on DVE
                tA = tabA[:, tloc:tloc + 512]
                tB = tabB[:, tloc:tloc + 512]
                for m in range(5):
                    qe = qes[m]
                    sw = pool.tile([128, 512], f32, tag="sw", bufs=1, name="sw")
                    nc.vector.tensor_copy(sw[0:64, :], qe[64:128, :])
                    nc.vector.tensor_copy(sw[64:128, :], qe[0:64, :])
                    mm = pool.tile([128, 512], f32, tag="mm", bufs=1, name="mm")
                    nc.vector.tensor_mul(mm[:], sw[:], tB)
                    tt = pool.tile([128, 512], f32, tag="tt", bufs=1, name="tt")
                    nc.vector.tensor_mul(tt[:], qe[:], tA)
                    if m < HPC:
                        ro = pool.tile([128, 512], f32r, tag="ro", bufs=2, name="ro")
                        nc.vector.tensor_add(ro[:], tt[:], mm[:])
                        nc.sync.dma_start(qTd[m * 128:(m + 1) * 128, n * 512:(n + 1) * 512], ro[:])
                    else:
                        nc.vector.tensor_add(KTb[b][:, tloc:tloc + 512], tt[:], mm[:])
                if n == 3:
                    # qTd rows for head 0 of batch 0 are complete: prefetch
                    # the first attention Q tile while phase A continues
                    nc.sync.dma_start(qt0[:], qTd[0:128, 0:T])

        # ---------------- Phase B: attention (ctx -> SBUF bf16) --------
        # Pool-open order controls space reuse (stack allocator): the
        # attention-critical pools (sxp/smp/qtp) open first so they land in
        # the weight pool's space, whose release only waits on phase A's
        # matmuls. ctx/wo open after and overlap the eviction pool, whose
        # release waits on the final rope DMAs — but their first use is well
        # into phase B. All SBUF pools stay open until kernel end (no stack
        # pops mid-kernel); only psB releases before phase C's PSUM pool.
        sxp = s0.enter_context(tc.tile_pool(name="sxp", bufs=6))
        smp = s0.enter_context(tc.tile_pool(name="smp", bufs=2))
        qtp = s0.enter_context(tc.tile_pool(name="qtp", bufs=2))
        ctxp = s0.enter_context(tc.tile_pool(name="ctxp", bufs=1))
        ctx_sb = [ctxp.tile([128, NT], bf16, tag=f"ctx{h}", name=f"ctx{h}")
                  for h in range(HPC)]
        wop = s0.enter_context(tc.tile_pool(name="wop", bufs=1))
        wo_sb = wop.tile([128, HPC * D], bf16, tag="wo")

        # psB split: the normalize tiles (ctx accumulator + broadcast bank)
        # outlive the score/rowsum banks, so phase C's PSUM pool can allocate
        # as soon as the main attention banks release.
        psBt = s0.enter_context(tc.tile_pool(name="psBt", bufs=2, space="PSUM"))
        with ExitStack() as sbc:
            psB = sbc.enter_context(tc.tile_pool(name="psB", bufs=2, space="PSUM"))

            pending_norm = None

            def flush_norm():
                nonlocal pending_norm
                if pending_norm is None:
                    return
                ps_ctx_p, rs_p, h_p, b_p, qt_p = pending_norm
                # broadcast rs across partitions on the (otherwise idle)
                # GPSIMD engine -- no PE matmul, no DVE copy
                bcs = smp.tile([128, 512], f32, tag="bcs", name="bcs")
                nc.gpsimd.partition_broadcast(bcs[:], rs_p[:])
                nc.vector.tensor_mul(
                    ctx_sb[h_p][:, b_p * T + qt_p * 512: b_p * T + (qt_p + 1) * 512],
                    ps_ctx_p[:], bcs[:])
                pending_norm = None

            for b in range(B):
                for h in range(HPC):
                    if b == 0 and h == 0:
                        qt_t = qt0
                    else:
                        qt_t = qtp.tile([128, T], f32r, tag="qt")
                        nc.sync.dma_start(qt_t[:], qTd[h * 128:(h + 1) * 128, b * T:(b + 1) * T])
                    if b == 1:
                        # prefetch one Wo head-slab per head during the second
                        # batch; issuing at b==0 would block the in-order Sync
                        # queue behind the eviction pool's release and delay
                        # the next head's Q-tile load
                        nc.sync.dma_start(wo_sb[:, h * D:(h + 1) * D],
                                          wo[h * 128:(h + 1) * 128, :])
                    for qt in range(4):
                        ps_ctx = psBt.tile([128, 512], f32, tag="ctx")
                        ps_sm = psB.tile([1, 512], f32, tag="sm")
                        nk = 4 * qt + 4

                        def issue_st(kt):
                            off = max(0, (kt - 4 * qt) * 128)
                            ps_st = psB.tile([128, 512], f32, tag="st", bufs=3, name="ps_st")
                            nc.tensor.matmul(ps_st[:, off:],
                                             KTb[b][:, kt * 128:(kt + 1) * 128],
                                             qt_t[:, qt * 512 + off:(qt + 1) * 512],
                                             start=True, stop=True)
                            se = sxp.tile([128, 512], f32r, tag="se", name="se")
                            nc.scalar.activation(se[:, off:], ps_st[:, off:],
                                                 mybir.ActivationFunctionType.Exp,
                                                 scale=EXP_SCALE)
                            if kt >= 4 * qt:
                                nc.vector.tensor_mul(se[:, off:off + 128],
                                                     se[:, off:off + 128], tri_sb[:])
                            return se, off

                        se_q = [issue_st(0), issue_st(1)]
                        for kt in range(nk):
                            se_cur, off = se_q.pop(0)
                            if kt + 2 < nk:
                                se_q.append(issue_st(kt + 2))
                            nc.tensor.matmul(ps_ctx[:, off:],
                                             Vb[b][:, kt * 128:(kt + 1) * 128],
                                             se_cur[:, off:],
                                             start=(kt == 0), stop=(kt == nk - 1))
                            nc.tensor.matmul(ps_sm[0:1, off:], ones[:], se_cur[:, off:],
                                             start=(kt == 0), stop=(kt == nk - 1))
                            if kt == 2:
                                flush_norm()  # prev q-tile's normalize, PE already busy
                        # reciprocal immediately (fast approx), consume one tile later
                        rs_f = smp.tile([1, 512], f32, tag="rsf", name="rs_f")
                        nc.vector.reciprocal_approx_fast(out=rs_f[:], in_=ps_sm[:])
                        pending_norm = (ps_ctx, rs_f, h, b, qt)

        # psB (score/rowsum banks) released here; the final q-tile's
        # normalize uses only psBt and runs concurrently with phase C's start
        flush_norm()

        # ---------------- Phase C: output projection (SBUF bf16) ----
        with ExitStack() as sc:
            psC = sc.enter_context(tc.tile_pool(name="psC", bufs=4, space="PSUM"))
            obp = sc.enter_context(tc.tile_pool(name="obp", bufs=2))
            for m in range(NT // 128):
                ob = obp.tile([128, D], bf16, tag="ob")
                for n in range(D // 512):
                    pso = psC.tile([128, 512], f32, tag="oc")
                    for h in range(HPC):
                        nc.tensor.matmul(pso[:],
                                         ctx_sb[h][:, m * 128:(m + 1) * 128],
                                         wo_sb[:, h * D + n * 512: h * D + (n + 1) * 512],
                                         start=(h == 0), stop=(h == HPC - 1))
                    # alternate eviction engines so neither stalls the PE
                    if n % 2 == 0:
                        nc.scalar.copy(ob[:, n * 512:(n + 1) * 512], pso[:])
                    else:
                        nc.vector.tensor_copy(ob[:, n * 512:(n + 1) * 512], pso[:])
                    if m == NT // 128 - 1 and n % 2 == 1:
                        # split the final tile's output DMA so the kernel tail
                        # isn't one serial 1MB transfer after the last eviction
                        nc.sync.dma_start(
                            outp[m * 128:(m + 1) * 128, (n - 1) * 512:(n + 1) * 512],
                            ob[:, (n - 1) * 512:(n + 1) * 512])
                if m < NT // 128 - 1:
                    nc.sync.dma_start(outp[m * 128:(m + 1) * 128, :], ob[:])

    nc.compile()
    return nc


def _get_nc():
    if "nc" not in _NC_CACHE:
        _NC_CACHE["nc"] = _build_program()
    return _NC_CACHE["nc"]


def _rope_tables():
    j = np.arange(0, DH, 2, dtype=np.float32) / np.float32(DH)
    inv_freq = (np.float32(1.0) / (np.float32(ROPE_BASE) ** j)).astype(np.float32)
    t = np.arange(T, dtype=np.float32)
    freqs = np.outer(t, inv_freq).astype(np.float32)   # (T, 64)
    c = np.cos(freqs).astype(np.float32).T             # (64, T)
    s = np.sin(freqs).astype(np.float32).T
    A = np.vstack([c, c]).astype(np.float32)           # (128, T)
    Bt = np.vstack([-s, s]).astype(np.float32)
    return np.ascontiguousarray(A), np.ascontiguousarray(Bt)


def _tri_mask():
    p = np.arange(128)[:, None]
    f = np.arange(128)[None, :]
    return np.ascontiguousarray((p <= f).astype(np.float32))


def _build_in_maps(x, Wq, Wk, Wv, Wo):
    import ml_dtypes

    bf = ml_dtypes.bfloat16
    xT = np.ascontiguousarray(x.reshape(NT, D).T).astype(bf)
    A, Bt = _rope_tables()
    tri = _tri_mask()
    in_maps = []
    for g in range(8):
        in_maps.append({
            "xT": xT,
            "wq": np.ascontiguousarray(Wq[:, g * FPC:(g + 1) * FPC]).astype(bf),
            "wk": np.ascontiguousarray(Wk[:, g * DH:(g + 1) * DH]).astype(bf),
            "wv": np.ascontiguousarray(Wv[:, g * DH:(g + 1) * DH]).astype(bf),
            "wo": np.ascontiguousarray(
                Wo[g * FPC:(g + 1) * FPC, :]).astype(bf),
            "ropeA": A,
            "ropeB": Bt,
            "trid": _round_fp32r(tri),
        })
    return in_maps


def kernel(x, Wq, Wk, Wv, Wo):
    x = np.asarray(x, dtype=np.float32)
    Wq = np.asarray(Wq, dtype=np.float32)
    Wk = np.asarray(Wk, dtype=np.float32)
    Wv = np.asarray(Wv, dtype=np.float32)
    Wo = np.asarray(Wo, dtype=np.float32)

    nc = _get_nc()
    in_maps = _build_in_maps(x, Wq, Wk, Wv, Wo)

    res = run_bass_kernel_spmd(nc, in_maps, list(range(8)))
    acc = res.results[0]["outp"].astype(np.float32)
    for g in range(1, 8):
        acc = acc + res.results[g]["outp"].astype(np.float32)
    return np.ascontiguousarray(acc.reshape(B, T, D), dtype=np.float32)


# revision 36
# speedup vs baseline: 1.0311x; 1.0036x over previous
"""GroupedQueryAttention TRN2 Bass kernel, sharded over 8 NeuronCores.

Problem (hardcoded): B=2, T=2048, D=4096, 32 Q heads x 128, 8 KV groups x 128,
RoPE (base 5e5), causal, out = ctx @ Wo.

Sharding: core g owns Q heads 4g..4g+3 (Wq columns 512g:512g+512), KV group g
(Wk/Wv columns 128g:128g+128), and Wo rows 512g:512g+512 (row-parallel).
Each core computes a full-shape partial output (bf16); host sums the 8
partials in fp32.

Matmuls run in float32r (fp32 with 11-bit mantissa, full PE rate); the
attention context and output projection run in bf16 (same PE rate, half the
SBUF/DMA traffic). Inputs are pre-rounded to fp32r on host.

Versus the 1.14ms baseline (final ~0.82ms, all phases 97-99% PE-busy):
 - softmax reciprocal via reciprocal_approx_fast (DVE reciprocal is an
   iterative divide, 3.3us per q-tile, and stalled the PE every tile);
   the denominator broadcast runs on the idle GPSIMD engine
 - causal diagonal blocks narrowed: score/exp/AV/rowsum matmuls only cover
   valid query columns (N = 512-128r); single [128,128] triangle mask
 - x/Wq/Wk/Wv in bf16 (same PE rate as f32r, half the DMA); attention
   GEMMs stay f32r (bf16 moving operands measured slower there)
 - small DMAs batched into 3D-AP transfers: the Sync queue's ~0.6us
   per-issue cost, not HBM bandwidth, limited the startup
 - ctx kept in SBUF as bf16 (no DRAM round-trip), Wo in bf16, output
   partials written bf16; one 1MB output DMA per token tile
 - pool scoping tuned so no phase waits on a prior phase's stragglers
   (eviction/rope tiles of the last tile live in a never-released pool)
 - exp ACT table preloaded at kernel start; Wo and the first Q tile
   prefetched during earlier phases
"""
import sys
import numpy as np

for _p in ("/opt/trn_rl_repo", "/root/.axon_site", "/root/.axon_site/_ro/trn_rl_repo"):
    if _p not in sys.path:
        sys.path.append(_p)

from contextlib import ExitStack

import concourse.bass as bass
import concourse.tile as tile
from concourse import bacc, mybir
from concourse.bass_utils import run_bass_kernel_spmd
from concourse.masks import make_identity

B, T, D = 2, 2048, 4096
NH, NKV, DH = 32, 8, 128
HPC = NH // 8          # 4 q heads per core
FPC = HPC * DH         # 512 q features per core
ROPE_BASE = 500000.0
NT = B * T             # 4096 tokens
f32 = mybir.dt.float32
f32r = mybir.dt.float32r
bf16 = mybir.dt.bfloat16
EXP_SCALE = 1.0 / float(np.sqrt(DH))

_NC_CACHE = {}


def _round_fp32r(x):
    x = np.ascontiguousarray(x, dtype=np.float32)
    u = x.view(np.uint32)
    lsb = (u >> 12) & np.uint32(1)
    r = (u + np.uint32(0x7FF) + lsb) & np.uint32(0xFFFFF000)
    return r.view(np.float32)


def _build_program():
    nc = bacc.Bacc("TRN2", target_bir_lowering=False, debug=False)

    xT = nc.dram_tensor("xT", [D, NT], bf16, kind="ExternalInput").ap()
    wq = nc.dram_tensor("wq", [D, FPC], bf16, kind="ExternalInput").ap()
    wk = nc.dram_tensor("wk", [D, DH], bf16, kind="ExternalInput").ap()
    wv = nc.dram_tensor("wv", [D, DH], bf16, kind="ExternalInput").ap()
    wo = nc.dram_tensor("wo", [FPC, D], bf16, kind="ExternalInput").ap()
    ropeA = nc.dram_tensor("ropeA", [128, T], f32, kind="ExternalInput").ap()
    ropeB = nc.dram_tensor("ropeB", [128, T], f32, kind="ExternalInput").ap()
    trid = nc.dram_tensor("trid", [128, 128], f32r, kind="ExternalInput").ap()
    outp = nc.dram_tensor("outp", [NT, D], bf16, kind="ExternalOutput").ap()

    qTd = nc.dram_tensor("qTd", [FPC, NT], f32r).ap()     # roped Q^T, feature-major

    KC = D // 128  # 32 contraction chunks

    with tile.TileContext(nc) as tc, ExitStack() as s0:
        kvp = s0.enter_context(tc.tile_pool(name="kv", bufs=1))
        KTb = [kvp.tile([128, T], f32r, tag=f"KT{i}", name=f"KT{i}") for i in range(B)]
        Vb = [kvp.tile([128, T], f32r, tag=f"V{i}", name=f"V{i}") for i in range(B)]
        ident_f = kvp.tile([128, 128], f32, tag="ident_f")
        make_identity(nc, ident_f[:])
        ident = kvp.tile([128, 128], f32r, tag="ident")
        nc.vector.tensor_copy(ident[:], ident_f[:])
        ones_f = kvp.tile([128, 1], f32, tag="ones_f")
        nc.vector.memset(ones_f[:], 1.0)
        ones = kvp.tile([128, 1], f32r, tag="ones")
        nc.vector.tensor_copy(ones[:], ones_f[:])
        ones_row_f = kvp.tile([1, 128], f32, tag="ones_row_f")
        nc.vector.memset(ones_row_f[:], 1.0)
        ones_row = kvp.tile([1, 128], f32r, tag="ones_row")
        nc.vector.tensor_copy(ones_row[:], ones_row_f[:])
        tri_sb = kvp.tile([128, 128], f32r, tag="tri")
        # first Q-head tile, prefetched during phase A (n==3) so attention
        # starts without a DMA stall
        qt0 = kvp.tile([128, T], f32r, tag="qt0")
        # preload the exp table set so the first attention exp doesn't pay
        # the ~2.7us ACT_TABLE_LOAD mid-kernel
        warm = kvp.tile([1, 2], f32, tag="warm")
        nc.vector.memset(warm[:], 0.0)
        nc.scalar.activation(warm[0:1, 0:1], warm[0:1, 1:2],
                             mybir.ActivationFunctionType.Exp, scale=1.0)
        # warm up the GPSIMD ext-isa library too: the first
        # partition_broadcast otherwise pays a ~6us IRAM load mid-attention
        wbc = kvp.tile([128, 2], f32, tag="wbc")
        nc.gpsimd.partition_broadcast(wbc[:], warm[0:1, :])

        # ---------------- Phase A: projections + RoPE -----------------
        # Eviction/rope tiles for the LAST n-tile live in a pool that stays
        # open to kernel end: phase B's pools then never wait on the final
        # rope chain (the A-scoped pools' releases would otherwise serialize
        # phase B behind ~9us of trailing DVE work).
        evt = s0.enter_context(tc.tile_pool(name="evt", bufs=1))
        with ExitStack() as sa:
            wp = sa.enter_context(tc.tile_pool(name="wts", bufs=1))
            wq_sb = wp.tile([128, KC * FPC], bf16, tag="wq")
            wk_sb = wp.tile([128, KC * DH], bf16, tag="wk")
            wv_sb = wp.tile([128, KC * DH], bf16, tag="wv")
            # rope tables in the never-released pool: the last n-tile's rope
            # reads them, and they must not pin the weight pool's release
            tabA = evt.tile([128, T], f32, tag="tabA")
            tabB = evt.tile([128, T], f32, tag="tabB")

            SLAB = 4
            NSLAB = KC // SLAB

            def load_w_slab(s):
                # one batched 3D-AP DMA per weight: DMA *issue* time on the
                # Sync queue (~0.6us each) is the startup bottleneck, not HBM
                k0, k1 = s * SLAB, (s + 1) * SLAB
                nc.sync.dma_start(
                    wq_sb[:, k0 * FPC:k1 * FPC],
                    wq[k0 * 128:k1 * 128, :].rearrange("(j p) c -> p j c", p=128))
                nc.sync.dma_start(
                    wk_sb[:, k0 * DH:k1 * DH],
                    wk[k0 * 128:k1 * 128, :].rearrange("(j p) c -> p j c", p=128))
                nc.sync.dma_start(
                    wv_sb[:, k0 * DH:k1 * DH],
                    wv[k0 * 128:k1 * 128, :].rearrange("(j p) c -> p j c", p=128))

            xsp = sa.enter_context(tc.tile_pool(name="xs", bufs=4))
            evp = sa.enter_context(tc.tile_pool(name="ev", bufs=1))
            psA = sa.enter_context(tc.tile_pool(name="psA", bufs=1, space="PSUM"))

            def stationary(m, k):
                if m < HPC:
                    return wq_sb[:, k * FPC + m * 128: k * FPC + (m + 1) * 128]
                if m == HPC:
                    return wk_sb[:, k * DH:(k + 1) * DH]
                return wv_sb[:, k * DH:(k + 1) * DH]

            pending_vt = None

            def flush_vt():
                nonlocal pending_vt
                if pending_vt is None:
                    return
                vt_p, n_p = pending_vt
                b_p = n_p // 4
                for i in range(4):
                    ptr = psA.tile([128, 128], f32r, tag="tr", bufs=2, name="ptr")
                    nc.tensor.transpose(ptr[:], vt_p[:, i * 128:(i + 1) * 128], ident[:])
                    c_local = 4 * (n_p % 4) + i
                    nc.scalar.copy(Vb[b_p][:, c_local * 128:c_local * 128 + 128], ptr[:])
                pending_vt = None

            for n in range(NT // 512):
                b, tloc = n // 4, 512 * (n % 4)
                ps = [psA.tile([128, 512], f32, tag=f"ps{m}", name=f"ps{m}")
                      for m in range(6)]
                for s in range(NSLAB):
                    if n == 0:
                        load_w_slab(s)
                    xsl = xsp.tile([128, SLAB * 512], bf16, tag="xs", name="xsl")
                    nc.sync.dma_start(
                        xsl[:],
                        xT[s * SLAB * 128:(s + 1) * SLAB * 128,
                           n * 512:(n + 1) * 512].rearrange(
                               "(j p) c -> p j c", p=128))
                    if n == 0 and s == 0:
                        # deferred setup loads: not needed until ~45us in
                        nc.sync.dma_start(tri_sb[:], trid)
                        nc.sync.dma_start(tabA[:], ropeA)
                        nc.sync.dma_start(tabB[:], ropeB)
                    for m in range(6):
                        for j in range(SLAB):
                            k = s * SLAB + j
                            nc.tensor.matmul(ps[m][:], stationary(m, k),
                                             xsl[:, j * 512:(j + 1) * 512],
                                             start=(k == 0), stop=(k == KC - 1))
                    if s == 0:
                        flush_vt()   # prev n-tile's V transposes, PE already warm
                # evict: ACT copies free PSUM banks at ACT pace; on the last
                # n-tile, flush V first and split evictions across ACT/DVE so
                # phase B's PSUM banks free as early as possible
                last = n == NT // 512 - 1
                pool = evt if last else evp
                qes = []
                if last:
                    vt = pool.tile([128, 512], f32r, tag="vt", bufs=1, name="vt")
                    nc.scalar.copy(vt[:], ps[5][:])
                    pending_vt = (vt, n)
                    flush_vt()
                    for m in range(5):
                        qe = pool.tile([128, 512], f32, tag="qe", bufs=5, name=f"qe{m}")
                        if m % 2 == 1:
                            nc.vector.tensor_copy(qe[:], ps[m][:])
                        else:
                            nc.scalar.copy(qe[:], ps[m][:])
                        qes.append(qe)
                else:
                    for m in range(5):
                        qe = pool.tile([128, 512], f32, tag="qe", bufs=6, name=f"qe{m}")
                        nc.scalar.copy(qe[:], ps[m][:])
                        qes.append(qe)
                    vt = pool.tile([128, 512], f32r, tag="vt", bufs=2, name="vt")
                    nc.scalar.copy(vt[:], ps[5][:])
                    pending_vt = (vt, n)
                # rope chains on DVE
                tA = tabA[:, tloc:tloc + 512]
                tB = tabB[:, tloc:tloc + 512]
                for m in range(5):
                    qe = qes[m]
                    sw = pool.tile([128, 512], f32, tag="sw", bufs=1, name="sw")
                    nc.vector.tensor_copy(sw[0:64, :], qe[64:128, :])
                    nc.vector.tensor_copy(sw[64:128, :], qe[0:64, :])
                    mm = pool.tile([128, 512], f32, tag="mm", bufs=1, name="mm")
                    nc.vector.tensor_mul(mm[:], sw[:], tB)
                    tt = pool.tile([128, 512], f32, tag="tt", bufs=1, name="tt")
                    nc.vector.tensor_mul(tt[:], qe[:], tA)
                    if m < HPC:
                        ro = pool.tile([128, 512], f32r, tag="ro", bufs=2, name="ro")
                        nc.vector.tensor_add(ro[:], tt[:], mm[:])
                        nc.sync.dma_start(qTd[m * 128:(m + 1) * 128, n * 512:(n + 1) * 512], ro[:])
                    else:
                        nc.vector.tensor_add(KTb[b][:, tloc:tloc + 512], tt[:], mm[:])
                if n == 3:
                    # qTd rows for head 0 of batch 0 are complete: prefetch
                    # the first attention Q tile while phase A continues
                    nc.sync.dma_start(qt0[:], qTd[0:128, 0:T])

        # ---------------- Phase B: attention (ctx -> SBUF bf16) --------
        # Pool-open order controls space reuse (stack allocator): the
        # attention-critical pools (sxp/smp/qtp) open first so they land in
        # the weight pool's space, whose release only waits on phase A's
        # matmuls. ctx/wo open after and overlap the eviction pool, whose
        # release waits on the final rope DMAs — but their first use is well
        # into phase B. All SBUF pools stay open until kernel end (no stack
        # pops mid-kernel); only psB releases before phase C's PSUM pool.
        sxp = s0.enter_context(tc.tile_pool(name="sxp", bufs=6))
        smp = s0.enter_context(tc.tile_pool(name="smp", bufs=2))
        qtp = s0.enter_context(tc.tile_pool(name="qtp", bufs=2))
        ctxp = s0.enter_context(tc.tile_pool(name="ctxp", bufs=1))
        ctx_sb = [ctxp.tile([128, NT], bf16, tag=f"ctx{h}", name=f"ctx{h}")
                  for h in range(HPC)]
        wop = s0.enter_context(tc.tile_pool(name="wop", bufs=1))
        wo_sb = wop.tile([128, HPC * D], bf16, tag="wo")

        # psB split: the normalize tiles (ctx accumulator + broadcast bank)
        # outlive the score/rowsum banks, so phase C's PSUM pool can allocate
        # as soon as the main attention banks release.
        psBt = s0.enter_context(tc.tile_pool(name="psBt", bufs=2, space="PSUM"))
        with ExitStack() as sbc:
            psB = sbc.enter_context(tc.tile_pool(name="psB", bufs=2, space="PSUM"))

            pending_norm = None

            def flush_norm():
                nonlocal pending_norm
                if pending_norm is None:
                    return
                ps_ctx_p, rs_p, h_p, b_p, qt_p = pending_norm
                # broadcast rs across partitions on the (otherwise idle)
                # GPSIMD engine -- no PE matmul, no DVE copy
                bcs = smp.tile([128, 512], f32, tag="bcs", name="bcs")
                nc.gpsimd.partition_broadcast(bcs[:], rs_p[:])
                nc.vector.tensor_mul(
                    ctx_sb[h_p][:, b_p * T + qt_p * 512: b_p * T + (qt_p + 1) * 512],
                    ps_ctx_p[:], bcs[:])
                pending_norm = None

            for b in range(B):
                for h in range(HPC):
                    if b == 0 and h == 0:
                        qt_t = qt0
                    else:
                        qt_t = qtp.tile([128, T], f32r, tag="qt")
                        nc.sync.dma_start(qt_t[:], qTd[h * 128:(h + 1) * 128, b * T:(b + 1) * T])
                    if b == 1:
                        # prefetch one Wo head-slab per head during the second
                        # batch; issuing at b==0 would block the in-order Sync
                        # queue behind the eviction pool's release and delay
                        # the next head's Q-tile load
                        nc.sync.dma_start(wo_sb[:, h * D:(h + 1) * D],
                                          wo[h * 128:(h + 1) * 128, :])
                    for qt in range(4):
                        ps_ctx = psBt.tile([128, 512], f32, tag="ctx")
                        ps_sm = psB.tile([1, 512], f32, tag="sm")
                        nk = 4 * qt + 4

                        def issue_st(kt):
                            off = max(0, (kt - 4 * qt) * 128)
                            ps_st = psB.tile([128, 512], f32, tag="st", bufs=4, name="ps_st")
                            nc.tensor.matmul(ps_st[:, off:],
                                             KTb[b][:, kt * 128:(kt + 1) * 128],
                                             qt_t[:, qt * 512 + off:(qt + 1) * 512],
                                             start=True, stop=True)
                            se = sxp.tile([128, 512], f32r, tag="se", name="se")
                            nc.scalar.activation(se[:, off:], ps_st[:, off:],
                                                 mybir.ActivationFunctionType.Exp,
                                                 scale=EXP_SCALE)
                            if kt >= 4 * qt:
                                nc.vector.tensor_mul(se[:, off:off + 128],
                                                     se[:, off:off + 128], tri_sb[:])
                            return se, off

                        se_q = [issue_st(0), issue_st(1), issue_st(2)]
                        for kt in range(nk):
                            se_cur, off = se_q.pop(0)
                            if kt + 3 < nk:
                                se_q.append(issue_st(kt + 3))
                            nc.tensor.matmul(ps_ctx[:, off:],
                                             Vb[b][:, kt * 128:(kt + 1) * 128],
                                             se_cur[:, off:],
                                             start=(kt == 0), stop=(kt == nk - 1))
                            nc.tensor.matmul(ps_sm[0:1, off:], ones[:], se_cur[:, off:],
                                             start=(kt == 0), stop=(kt == nk - 1))
                            if kt == 2:
                                flush_norm()  # prev q-tile's normalize, PE already busy
                        # reciprocal immediately (fast approx), consume one tile later
                        rs_f = smp.tile([1, 512], f32, tag="rsf", name="rs_f")
                        nc.vector.reciprocal_approx_fast(out=rs_f[:], in_=ps_sm[:])
                        pending_norm = (ps_ctx, rs_f, h, b, qt)

        # psB (score/rowsum banks) released here; the final q-tile's
        # normalize uses only psBt and runs concurrently with phase C's start
        flush_norm()

        # ---------------- Phase C: output projection (SBUF bf16) ----
        with ExitStack() as sc:
            psC = sc.enter_context(tc.tile_pool(name="psC", bufs=4, space="PSUM"))
            obp = sc.enter_context(tc.tile_pool(name="obp", bufs=2))
            for m in range(NT // 128):
                ob = obp.tile([128, D], bf16, tag="ob")
                for n in range(D // 512):
                    pso = psC.tile([128, 512], f32, tag="oc")
                    for h in range(HPC):
                        nc.tensor.matmul(pso[:],
                                         ctx_sb[h][:, m * 128:(m + 1) * 128],
                                         wo_sb[:, h * D + n * 512: h * D + (n + 1) * 512],
                                         start=(h == 0), stop=(h == HPC - 1))
                    # alternate eviction engines so neither stalls the PE
                    if n % 2 == 0:
                        nc.scalar.copy(ob[:, n * 512:(n + 1) * 512], pso[:])
                    else:
                        nc.vector.tensor_copy(ob[:, n * 512:(n + 1) * 512], pso[:])
                    if m == NT // 128 - 1 and n % 2 == 1:
                        # split the final tile's output DMA so the kernel tail
                        # isn't one serial 1MB transfer after the last eviction
                        nc.sync.dma_start(
                            outp[m * 128:(m + 1) * 128, (n - 1) * 512:(n + 1) * 512],
                            ob[:, (n - 1) * 512:(n + 1) * 512])
                if m < NT // 128 - 1:
                    nc.sync.dma_start(outp[m * 128:(m + 1) * 128, :], ob[:])

    nc.compile()
    return nc


def _get_nc():
    if "nc" not in _NC_CACHE:
        _NC_CACHE["nc"] = _build_program()
    return _NC_CACHE["nc"]


def _rope_tables():
    j = np.arange(0, DH, 2, dtype=np.float32) / np.float32(DH)
    inv_freq = (np.float32(1.0) / (np.float32(ROPE_BASE) ** j)).astype(np.float32)
    t = np.arange(T, dtype=np.float32)
    freqs = np.outer(t, inv_freq).astype(np.float32)   # (T, 64)
    c = np.cos(freqs).astype(np.float32).T             # (64, T)
    s = np.sin(freqs).astype(np.float32).T
    A = np.vstack([c, c]).astype(np.float32)           # (128, T)
    Bt = np.vstack([-s, s]).astype(np.float32)
    return np.ascontiguousarray(A), np.ascontiguousarray(Bt)


def _tri_mask():
    p = np.arange(128)[:, None]
    f = np.arange(128)[None, :]
    return np.ascontiguousarray((p <= f).astype(np.float32))


def _build_in_maps(x, Wq, Wk, Wv, Wo):
    import ml_dtypes

    bf = ml_dtypes.bfloat16
    xT = np.ascontiguousarray(x.reshape(NT, D).T).astype(bf)
    A, Bt = _rope_tables()
    tri = _tri_mask()
    in_maps = []
    for g in range(8):
        in_maps.append({
            "xT": xT,
            "wq": np.ascontiguousarray(Wq[:, g * FPC:(g + 1) * FPC]).astype(bf),
            "wk": np.ascontiguousarray(Wk[:, g * DH:(g + 1) * DH]).astype(bf),
            "wv": np.ascontiguousarray(Wv[:, g * DH:(g + 1) * DH]).astype(bf),
            "wo": np.ascontiguousarray(
                Wo[g * FPC:(g + 1) * FPC, :]).astype(bf),
            "ropeA": A,
            "ropeB": Bt,
            "trid": _round_fp32r(tri),
        })
    return in_maps


def kernel(x, Wq, Wk, Wv, Wo):
    x = np.asarray(x, dtype=np.float32)
    Wq = np.asarray(Wq, dtype=np.float32)
    Wk = np.asarray(Wk, dtype=np.float32)
    Wv = np.asarray(Wv, dtype=np.float32)
    Wo = np.asarray(Wo, dtype=np.float32)

    nc = _get_nc()
    in_maps = _build_in_maps(x, Wq, Wk, Wv, Wo)

    res = run_bass_kernel_spmd(nc, in_maps, list(range(8)))
    acc = res.results[0]["outp"].astype(np.float32)
    for g in range(1, 8):
        acc = acc + res.results[g]["outp"].astype(np.float32)
    return np.ascontiguousarray(acc.reshape(B, T, D), dtype=np.float32)
